# revision 1
# baseline (speedup 1.0000x reference)
"""ComplexAttentionLayer Trainium2 kernel, v2 (8-core data-parallel).

Math (per token t, head h; E=64; the per-head feature dim is 1, so scores
are outer products over the E axis):
  abs2[l,s] = 0.5*(Gp[l]*Hm[s] + Gm[l]*Hp[s]),  Gp=(qr+qi)^2, Gm=(qr-qi)^2,
                                                Hp=(kr+ki)^2, Hm=(kr-ki)^2
  attn = softmax(sqrt(abs2), axis=s)   (unnormalized exp; scores >= 0 and
         bounded far below f32 exp overflow, so no max subtraction)
  out[l] = sum_s attn[l,s] * v[s]

v2 structure: the score outer products and the attn@v contraction both run
on the tensor engine.  Tokens are processed in pairs (p, p+64) of a
128-token tile; per (token-tile, head, 32-pair chunk):
  - abs2: lhsT = H-staging [K=128 (4 used rows), (blk,s)=128], rhs =
    G-staging rows 0-3 = (GpA, GmA, GpB, GmB) -> psum [(blk,s), l]
  - scalar engine: E = exp(exp(0.5*ln(0.5*S)))  (single ln/exp table set)
  - contraction: lhsT = E-pair [(blk,s), l], rhs = V-staging [(blk,s), 6]
    with columns (vrA, viA, 1A, vrB, viB, 1B) -> psum [l, (pair, 6)]
    landing s.t. partitions = h-major output index d' = h*64+l
  - normalize with reciprocal of the ones-column sums, write transposed
    V directly in the output projection's lhsT layout.
sqrt is computed as exp(0.5*ln(x)) so the scalar engine never switches
activation-table sets.
"""

import numpy as np

import concourse.bass as bass
import concourse.tile as tile
from concourse import bacc, mybir
from concourse.bass_utils import run_bass_kernel_spmd
from concourse.masks import make_identity

AF = mybir.ActivationFunctionType
ALU = mybir.AluOpType
F32 = mybir.dt.float32
BF16 = mybir.dt.bfloat16

B, L, D, H = 4, 1024, 512, 8
E = D // H  # 64
NCORES = 8
T = B * L // NCORES  # 512 tokens per core
PT = 128             # tokens per tile
NTT = T // PT        # 4 token tiles per core
KT = D // 128        # 4 k-tiles per weight
NPAIR = 64           # token pairs (p, p+64) per tile
CH = 32              # pairs per staging chunk
HC = 8               # pairs per abs2-psum sub chunk


def _patch_act_tables():
    """Keep Ln/Exp/Copy/Identity only in natural_log_exp_and_others so the
    table-load pass picks the one set covering all our activations (one
    ACT_TABLE_LOAD instead of ping-ponging between per-function sets)."""
    from concourse import hw_specs
    orig = hw_specs.get_activation_tables
    AFT = mybir.ActivationFunctionType
    ours = {AFT.Ln, AFT.Exp, AFT.Copy, AFT.Identity}

    def patched(module_arch):
        tabs = orig(module_arch)
        if "natural_log_exp_and_others" in tabs:
            for name, fns in tabs.items():
                if name != "natural_log_exp_and_others":
                    tabs[name] = fns - ours
        return tabs

    bacc.get_activation_tables = patched


def _build_module():
    nc = bacc.Bacc()

    xT = {}
    for nm in ("q_r", "q_i", "k_r", "k_i", "v_r", "v_i"):
        xT[nm] = nc.declare_dram_parameter(f"x_{nm}_T", [D, T], BF16, isOutput=False)
    w = {}
    for p in ("q", "k", "v", "o"):
        for c in ("r", "i", "in"):  # r = w_r.T, i = w_i.T, in = -w_i.T
            w[p, c] = nc.declare_dram_parameter(f"w_{p}_{c}", [D, D], BF16,
                                                isOutput=False)
    bias = {}
    for p in ("q", "k", "v", "o"):
        for c in ("r", "i"):  # r: br-bi, i: br+bi
            bias[p, c] = nc.declare_dram_parameter(f"b_{p}_{c}", [1, D], BF16,
                                                   isOutput=False)
    out_r = nc.declare_dram_parameter("out_r", [T, D], F32, isOutput=True)
    out_i = nc.declare_dram_parameter("out_i", [T, D], F32, isOutput=True)

    with tile.TileContext(nc) as tc:
        with (
            tc.tile_pool(name="const", bufs=1) as const_pool,
            tc.tile_pool(name="xin", bufs=2) as x_pool,
            tc.tile_pool(name="wgt", bufs=2) as w_pool,
            tc.tile_pool(name="score", bufs=2) as score_pool,
            tc.tile_pool(name="acc", bufs=2) as acc_pool,
            tc.tile_pool(name="evac", bufs=3) as evac_pool,
            tc.tile_pool(name="psum", bufs=2, space="PSUM") as psum_pool,
            tc.tile_pool(name="psum_a", bufs=2, space="PSUM") as psum_a_pool,
            tc.tile_pool(name="psum_c", bufs=2, space="PSUM") as psum_c_pool,
        ):
            ident = const_pool.tile([128, 128], F32, tag="ident")
            make_identity(nc, ident[:])
            ones_row = const_pool.tile([1, 128], BF16, tag="ones")
            nc.gpsimd.memset(ones_row[:], 1.0)

            bs = {}
            for key, dram in bias.items():
                t = const_pool.tile([1, D], BF16, name=f"b_{key[0]}_{key[1]}",
                                    tag=f"b_{key[0]}_{key[1]}")
                nc.sync.dma_start(t[:], dram[:])
                bs[key] = t

            def load_kxn(pool, dram, tag, n, eng=None):
                t = pool.tile([128, KT, n], BF16, name=tag, tag=tag)
                (eng or nc.sync).dma_start(
                    t[:], dram[:].rearrange("(k p) n -> p k n", p=128))
                return t

            # G/H/v for the whole core, h-major free layout: d' = h*64 + e
            ghv = {nm: const_pool.tile([PT, NTT, D],
                                       F32 if nm[0] == "v" else BF16,
                                       name=f"ghv_{nm}", tag=f"ghv_{nm}")
                   for nm in ("Gp", "Gm", "Hp", "Hm", "vr", "vi")}

            def hmaj(ap):
                # natural [p, (l h)] view -> h-major [p, l, h] target strides
                return ap.rearrange("p (h l) -> p l h", l=E)

            def nat(ap):
                return ap.rearrange("p (l h) -> p l h", h=H)

            def cproj(wset, xr_t, xi_t, tt):
                """complex linear on token tile tt -> (psum_yr, psum_yi)"""
                ts = bass.ts(tt, PT)
                yr = psum_pool.tile([PT, D], F32, tag="mm")
                yi = psum_pool.tile([PT, D], F32, tag="mm")
                for k in range(KT):
                    nc.tensor.matmul(yr[:], xr_t[:, k, ts], wset["r"][:, k, :],
                                     start=(k == 0), stop=False)
                for k in range(KT):
                    nc.tensor.matmul(yr[:], xi_t[:, k, ts], wset["in"][:, k, :],
                                     start=False, stop=False)
                nc.tensor.matmul(yr[:], ones_row[:], wset["br"][:],
                                 start=False, stop=True)
                for k in range(KT):
                    nc.tensor.matmul(yi[:], xi_t[:, k, ts], wset["r"][:, k, :],
                                     start=(k == 0), stop=False)
                for k in range(KT):
                    nc.tensor.matmul(yi[:], xr_t[:, k, ts], wset["i"][:, k, :],
                                     start=False, stop=False)
                nc.tensor.matmul(yi[:], ones_row[:], wset["bi"][:],
                                 start=False, stop=True)
                return yr, yi

            # ---- phase 1: q/k/v projections -> G/H/v (h-major) ----
            for p, (sum_nm, dif_nm) in (("q", ("Gp", "Gm")),
                                        ("k", ("Hp", "Hm")),
                                        ("v", ("vr", "vi"))):
                eng = nc.sync if p == "q" else nc.gpsimd
                xr_t = load_kxn(x_pool, xT[f"{p}_r"], "xr", T, eng)
                xi_t = load_kxn(x_pool, xT[f"{p}_i"], "xi", T, eng)
                wset = {c: load_kxn(w_pool, w[p, c], f"w{c}", D, eng)
                        for c in ("r", "i", "in")}  # noqa
                wset["br"] = bs[p, "r"]
                wset["bi"] = bs[p, "i"]
                for tt in range(NTT):
                    yr, yi = cproj(wset, xr_t, xi_t, tt)
                    if p == "v":
                        nc.scalar.copy(hmaj(ghv["vr"][:, tt, :]), nat(yr[:]))
                        nc.scalar.copy(hmaj(ghv["vi"][:, tt, :]), nat(yi[:]))
                    else:
                        yr_s = evac_pool.tile([PT, D], F32, tag="evac")
                        nc.vector.tensor_copy(yr_s[:], yr[:])
                        yi_s = evac_pool.tile([PT, D], F32, tag="evac")
                        nc.vector.tensor_copy(yi_s[:], yi[:])
                        tp = evac_pool.tile([PT, D], F32, tag="evac")
                        nc.vector.tensor_add(tp[:], yr_s[:], yi_s[:])
                        nc.vector.tensor_mul(hmaj(ghv[sum_nm][:, tt, :]),
                                             nat(tp[:]), nat(tp[:]))
                        tm = evac_pool.tile([PT, D], F32, tag="evac")
                        nc.vector.tensor_sub(tm[:], yr_s[:], yi_s[:])
                        nc.vector.tensor_mul(hmaj(ghv[dif_nm][:, tt, :]),
                                             nat(tm[:]), nat(tm[:]))

            # ---- phase 2: attention (PE outer products + PE contraction) --
            # output-projection operand, h-major: row d' = h*64+l ->
            # partition (h%2)*64+l, plane h//2; col = token
            VrT = const_pool.tile([128, KT, T], BF16, tag="VrT")
            ViT = const_pool.tile([128, KT, T], BF16, tag="ViT")

            # staging buffers (manual ping-pong)
            hst = [const_pool.tile([128, CH * 128], BF16, name=f"hst{i}",
                                   tag=f"hst{i}") for i in range(2)]
            gst = [const_pool.tile([128, CH * E], BF16, name=f"gst{i}",
                                   tag=f"gst{i}") for i in range(2)]
            vst = [const_pool.tile([128, CH * 6], BF16, name=f"vst{i}",
                                   tag=f"vst{i}") for i in range(2)]
            for i in range(2):
                nc.gpsimd.memset(hst[i][:], 0.0)
                nc.gpsimd.memset(gst[i][:], 0.0)
                nc.gpsimd.memset(vst[i][:], 0.0)
                on6 = vst[i][:].rearrange("p (j c) -> p j c", c=6)
                nc.gpsimd.memset(on6[0:64, :, 2], 1.0)
                nc.gpsimd.memset(on6[64:128, :, 5], 1.0)

            chunk_idx = 0
            for tt in range(NTT):
                for h in range(H):
                    hs = slice(h * E, (h + 1) * E)
                    base = 64 * (h % 2)
                    # V transpose for this (tt, h) via regular matmul
                    # against the identity (transpose-mode can't write at
                    # partition 64): out = v.T @ I.  cols 0:64 = vr,
                    # 64:128 = vi; rows 0:64 = A tokens, 64:128 = B tokens
                    vtr = psum_c_pool.tile([128, 128], F32, tag="ctr")
                    nc.tensor.matmul(vtr[0:64, 0:64],
                                     ghv["vr"][0:64, tt, hs],
                                     ident[0:64, 0:64])
                    nc.tensor.matmul(vtr[64:128, 0:64],
                                     ghv["vr"][64:128, tt, hs],
                                     ident[64:128, 64:128])
                    nc.tensor.matmul(vtr[0:64, 64:128],
                                     ghv["vi"][0:64, tt, hs],
                                     ident[0:64, 0:64])
                    nc.tensor.matmul(vtr[64:128, 64:128],
                                     ghv["vi"][64:128, tt, hs],
                                     ident[64:128, 64:128])

                    ctr = psum_c_pool.tile([128, NPAIR * 6], F32, tag="bigctr")
                    Et_full = score_pool.tile([128, NPAIR * E], BF16, tag="Et")

                    for c in range(2):  # staging chunks of CH=32 pairs
                        sl32 = slice(c * CH, (c + 1) * CH)
                        sl32b = slice(64 + c * CH, 64 + (c + 1) * CH)
                        hb = hst[chunk_idx % 2]
                        gb = gst[chunk_idx % 2]
                        vb = vst[chunk_idx % 2]
                        chunk_idx += 1
                        # G staging rows 0-3 = GpA, GmA, GpB, GmB
                        for r, (nm, ts_) in enumerate(
                                (("Gp", sl32), ("Gm", sl32),
                                 ("Gp", sl32b), ("Gm", sl32b))):
                            dv = gb[r:r + 1, 0:CH * E]
                            nc.sync.dma_start(dv, ghv[nm][ts_, tt, hs])
                        # H staging rows 0-3 = HmA, HpA (cols 0:64),
                        #                      HmB, HpB (cols 64:128)
                        for r, (nm, ts_, half) in enumerate(
                                (("Hm", sl32, 0), ("Hp", sl32, 0),
                                 ("Hm", sl32b, 1), ("Hp", sl32b, 1))):
                            dv = hb[r:r + 1, :].rearrange(
                                "p (j two s) -> p j two s",
                                two=2, s=E)[:, :, half, :]
                            nc.sync.dma_start(dv, ghv[nm][ts_, tt, hs])
                        # V staging: cols 6j + (0,1,2 | 3,4,5) =
                        #            (vrA, viA, 1) | (vrB, viB, 1)
                        v6 = vb[:].rearrange("p (j c) -> p j c", c=6)
                        slv = slice(c * CH, (c + 1) * CH)
                        slvi = slice(64 + c * CH, 64 + (c + 1) * CH)
                        nc.vector.tensor_copy(v6[0:64, :, 0], vtr[0:64, slv])
                        nc.vector.tensor_copy(v6[0:64, :, 1], vtr[0:64, slvi])
                        nc.vector.tensor_copy(v6[64:128, :, 3],
                                              vtr[64:128, slv])
                        nc.vector.tensor_copy(v6[64:128, :, 4],
                                              vtr[64:128, slvi])

                        sA = score_pool.tile([128, CH * E], F32, tag="sA")
                        for hc in range(CH // HC):  # abs2 sub chunks of HC
                            ab = psum_a_pool.tile([128, HC * E], F32,
                                                  tag="abs2")
                            for jl in range(HC):
                                j = hc * HC + jl
                                nc.tensor.matmul(
                                    ab[:, bass.ts(jl, E)],
                                    hb[:, bass.ts(j, 128)],
                                    gb[:, bass.ts(j, E)])
                            # PE result is 2*abs2; ln(0.5 * PE) = ln(abs2)
                            nc.scalar.activation(
                                sA[:, bass.ts(hc, HC * E)], ab[:],
                                AF.Ln, scale=0.5)
                        sB = score_pool.tile([128, CH * E], F32, tag="sB")
                        nc.scalar.activation(sB[:], sA[:], AF.Exp, scale=0.5)
                        nc.scalar.activation(
                            Et_full[:, bass.ts(c, CH * E)], sB[:], AF.Exp)

                        # contraction for this chunk
                        for jl in range(CH):
                            j = c * CH + jl
                            nc.tensor.matmul(
                                ctr[base:base + 64, bass.ts(j, 6)],
                                Et_full[:, bass.ts(j, E)],
                                vb[:, bass.ts(jl, 6)])

                    # normalize + write transposed V (bf16) for o-proj
                    c6 = ctr[base:base + 64, :].rearrange(
                        "p (j blk c) -> p j blk c", blk=2, c=3)
                    rcp = acc_pool.tile([64, 128], F32, tag="rcp")
                    nc.vector.reciprocal(rcp[:].rearrange(
                        "p (blk j) -> p j blk", blk=2), c6[:, :, :, 2])
                    # VrT view: partition base..base+64, plane h//2,
                    # cols (blk*64 + j) + tt*128
                    vr_dst = VrT[base:base + 64, h // 2,
                                 tt * PT:(tt + 1) * PT].rearrange(
                                     "p (blk j) -> p j blk", blk=2)
                    vi_dst = ViT[base:base + 64, h // 2,
                                 tt * PT:(tt + 1) * PT].rearrange(
                                     "p (blk j) -> p j blk", blk=2)
                    rcpv = rcp[:].rearrange("p (blk j) -> p j blk", blk=2)
                    nc.vector.tensor_mul(vr_dst, c6[:, :, :, 0], rcpv)
                    nc.vector.tensor_mul(vi_dst, c6[:, :, :, 1], rcpv)

            # ---- phase 3: output projection ----
            wo = {c: load_kxn(w_pool, w["o", c], f"w{c}", D, nc.gpsimd)
                  for c in ("r", "i", "in")}
            wo["br"] = bs["o", "r"]
            wo["bi"] = bs["o", "i"]
            for tt in range(NTT):
                ts = bass.ts(tt, PT)
                our = psum_pool.tile([PT, D], F32, tag="mm")
                oui = psum_pool.tile([PT, D], F32, tag="mm")
                for k in range(KT):
                    nc.tensor.matmul(our[:], VrT[:, k, ts], wo["r"][:, k, :],
                                     start=(k == 0), stop=False)
                for k in range(KT):
                    nc.tensor.matmul(our[:], ViT[:, k, ts], wo["in"][:, k, :],
                                     start=False, stop=False)
                nc.tensor.matmul(our[:], ones_row[:], wo["br"][:],
                                 start=False, stop=True)
                for k in range(KT):
                    nc.tensor.matmul(oui[:], ViT[:, k, ts], wo["r"][:, k, :],
                                     start=(k == 0), stop=False)
                for k in range(KT):
                    nc.tensor.matmul(oui[:], VrT[:, k, ts], wo["i"][:, k, :],
                                     start=False, stop=False)
                nc.tensor.matmul(oui[:], ones_row[:], wo["bi"][:],
                                 start=False, stop=True)

                sor = evac_pool.tile([PT, D], F32, tag="sor")
                soi = evac_pool.tile([PT, D], F32, tag="soi")
                nc.scalar.copy(sor[:], our[:])
                nc.scalar.copy(soi[:], oui[:])
                nc.gpsimd.dma_start(out_r[ts, :], sor[:])
                nc.gpsimd.dma_start(out_i[ts, :], soi[:])

    nc.compile()
    return nc


_NC_CACHE = None


def _get_module():
    global _NC_CACHE
    if _NC_CACHE is None:
        _patch_act_tables()
        _NC_CACHE = _build_module()
    return _NC_CACHE


def _prep_inputs(inputs):
    """host-side shard/layout prep -> list of 8 per-core input maps"""
    import ml_dtypes
    bf = ml_dtypes.bfloat16
    TT = B * L
    xs = {nm: np.ascontiguousarray(
        np.asarray(inputs[nm]).reshape(TT, D).T.astype(bf))
        for nm in ("q_r", "q_i", "k_r", "k_i", "v_r", "v_i")}
    # o-projection contracts over d in h-major order d' = h*64 + l:
    # permute the corresponding weight rows
    perm = np.empty(D, np.int64)
    for h in range(H):
        for l in range(E):
            perm[h * E + l] = l * H + h
    common = {}
    for p in ("q", "k", "v", "o"):
        wr = np.asarray(inputs[f"w{p}_r"]).astype(np.float32)
        wi = np.asarray(inputs[f"w{p}_i"]).astype(np.float32)
        br = np.asarray(inputs[f"b{p}_r"]).astype(np.float32)
        bi = np.asarray(inputs[f"b{p}_i"]).astype(np.float32)
        wrT = wr.T
        wiT = wi.T
        if p == "o":
            wrT = wrT[perm, :]
            wiT = wiT[perm, :]
        common[f"w_{p}_r"] = np.ascontiguousarray(wrT.astype(bf))
        common[f"w_{p}_i"] = np.ascontiguousarray(wiT.astype(bf))
        common[f"w_{p}_in"] = np.ascontiguousarray((-wiT).astype(bf))
        common[f"b_{p}_r"] = (br - bi).reshape(1, D).astype(bf)
        common[f"b_{p}_i"] = (br + bi).reshape(1, D).astype(bf)
    maps = []
    for c in range(NCORES):
        m = dict(common)
        sl = slice(c * T, (c + 1) * T)
        for nm, arr in xs.items():
            m[f"x_{nm}_T"] = np.ascontiguousarray(arr[:, sl])
        maps.append(m)
    return maps


def kernel(**inputs):
    nc = _get_module()
    maps = _prep_inputs(inputs)
    res = run_bass_kernel_spmd(nc, maps, core_ids=list(range(NCORES)))
    out_r = np.concatenate([res.results[c]["out_r"] for c in range(NCORES)],
                           axis=0).reshape(B, L, D)
    out_i = np.concatenate([res.results[c]["out_i"] for c in range(NCORES)],
                           axis=0).reshape(B, L, D)
    return out_r, out_i



# revision 13
# speedup vs baseline: 2.0834x; 2.0834x over previous
"""ComplexAttentionLayer Trainium2 kernel, v3 (8-core data-parallel).

Math (per token t, head h; E=64; per-head feature dim is 1, so scores are
outer products over the E axis):
  w[l,s]   = Gp[l]*Hm[s] + Gm[l]*Hp[s]       (= 2*abs2, PE outer products)
             Gp=(qr+qi)^2, Gm=(qr-qi)^2, Hp=(kr+ki)^2, Hm=(kr-ki)^2
  score    = sqrt(0.5*w)                      (ACT Sqrt table, exact)
  E        = exp(score)   via the Schraudolph bf16 bit trick on the DVE:
             bits16 = round(A2*score + 16256), A2 = 128/ln2; the bf16 with
             those bits is exp(score)*R(phi), R in [1, 1.0613] a mantissa
             sawtooth.  A second sample bits16+64 shifts the sawtooth phase
             by half a period (and multiplies by sqrt2); contracting
             E1 against V and E2 against V/sqrt2 in one accumulating PSUM
             group averages the two phases: residual error ~ +-0.8%.
  out[l]   = sum_s E[l,s] v[s] / sum_s E[l,s]  (PE per-token matmuls with a
             ones column for the denominator; DVE reciprocal+mul normalize)

The ACT engine runs ONLY the sqrt pass (one table set, loaded once); the
exp lives on the DVE at its 4x (2-byte) rate; abs2/contraction/projections
are PE matmuls; staging uses 8 flatten-DMAs per (tt, quarter) and V is
transposed with the XBAR dma_start_transpose.
"""

import math

import numpy as np

import concourse.bass as bass
import concourse.tile as tile
from concourse import bacc, mybir
from concourse.bass_utils import run_bass_kernel_spmd

AF = mybir.ActivationFunctionType
ALU = mybir.AluOpType
F32 = mybir.dt.float32
F16 = mybir.dt.float16
I16 = mybir.dt.int16
BF16 = mybir.dt.bfloat16

B, L, D, H = 4, 1024, 512, 8
E = D // H           # 64
NCORES = 8
T = B * L // NCORES  # 512 tokens per core
PT = 128             # tokens per tile
NTT = T // PT        # 4 token tiles per core
KT = D // 128        # 4 k-tiles per weight
NJQ = 4              # token quarters per tile
TQ = PT // NJQ       # 32 tokens per (tt, jq)
NPJ = TQ // 2        # 16 pairs per (tt, jq)

A2 = 128.0 / math.log(2.0)
SQ_SCALE = 0.5 * A2 * A2   # sqrt(SQ_SCALE*w) = A2*sqrt(0.5*w) = A2*score
BPRIME = 16256.0
INV_SQRT2 = 1.0 / math.sqrt(2.0)


def _build_module():
    nc = bacc.Bacc()

    xT = {}
    for nm in ("q_r", "q_i", "k_r", "k_i", "v_r", "v_i"):
        xT[nm] = nc.declare_dram_parameter(f"x_{nm}_T", [D, T], BF16, isOutput=False)
    w = {}
    for p in ("q", "k", "v", "o"):
        for c in ("r", "i", "in"):  # r = w_r.T, i = w_i.T, in = -w_i.T
            w[p, c] = nc.declare_dram_parameter(f"w_{p}_{c}", [D, D], BF16,
                                                isOutput=False)
    bias = {}
    for p in ("q", "k", "v", "o"):
        for c in ("r", "i"):  # r: br-bi, i: br+bi
            bias[p, c] = nc.declare_dram_parameter(f"b_{p}_{c}", [1, D], BF16,
                                                   isOutput=False)
    out_r = nc.declare_dram_parameter("out_r", [T, D], F32, isOutput=True)
    out_i = nc.declare_dram_parameter("out_i", [T, D], F32, isOutput=True)

    with tile.TileContext(nc) as tc:
        with (
            tc.tile_pool(name="const", bufs=1) as const_pool,
            tc.tile_pool(name="xin", bufs=1) as x_pool,
            tc.tile_pool(name="wgt", bufs=1) as w_pool,
            tc.tile_pool(name="stage", bufs=1) as stage_pool,
            tc.tile_pool(name="v2", bufs=2) as v2_pool,
            tc.tile_pool(name="v6", bufs=2) as v6_pool,
            tc.tile_pool(name="gh", bufs=2) as gh_pool,
            tc.tile_pool(name="evac", bufs=1) as evac_pool,
            tc.tile_pool(name="sco", bufs=2) as s_pool,
            tc.tile_pool(name="et", bufs=2) as et_pool,
            tc.tile_pool(name="nrm", bufs=2) as norm_pool,
            tc.tile_pool(name="ps", bufs=3, space="PSUM") as ps_pool,
            tc.tile_pool(name="psc", bufs=2, space="PSUM") as ctr_pool,
        ):
            ones_row = const_pool.tile([1, 128], BF16, tag="ones")
            nc.gpsimd.memset(ones_row[:], 1.0)

            ball = const_pool.tile([1, 8, D], BF16, tag="ball")
            bs = {}
            for i, (key, dram) in enumerate(sorted(bias.items())):
                nc.sync.dma_start(ball[:, i, :], dram[:])
                bs[key] = ball[:, i, :]


            # o-projection operands (h-major d' = h*64+l)
            VrT = const_pool.tile([128, KT, T], BF16, tag="VrT")
            ViT = const_pool.tile([128, KT, T], BF16, tag="ViT")

            def load_kxn(pool, dram, tag, n, eng=None):
                t = pool.tile([128, KT, n], BF16, name=tag, tag=tag)
                (eng or nc.gpsimd).dma_start(
                    t[:], dram[:].rearrange("(k p) n -> p k n", p=128))
                return t

            def cproj(wset, xr_t, xi_t, tt):
                """complex linear on token tile tt -> psum [128, 1024]
                (yr cols 0:512, yi cols 512:1024)"""
                ts = slice(0, PT)
                ps = ps_pool.tile([128, NPJ, E], F32, tag="ps")
                flat = ps[:].rearrange("p a b -> p (a b)")
                yr = flat[:, 0:D]
                yi = flat[:, D:2 * D]
                for k in range(KT):
                    nc.tensor.matmul(yr, xr_t[:, k, ts], wset["r"][:, k, :],
                                     start=(k == 0), stop=False)
                for k in range(KT):
                    nc.tensor.matmul(yr, xi_t[:, k, ts], wset["in"][:, k, :],
                                     start=False, stop=False)
                nc.tensor.matmul(yr, ones_row[:], wset["br"],
                                 start=False, stop=True)
                for k in range(KT):
                    nc.tensor.matmul(yi, xi_t[:, k, ts], wset["r"][:, k, :],
                                     start=(k == 0), stop=False)
                for k in range(KT):
                    nc.tensor.matmul(yi, xr_t[:, k, ts], wset["i"][:, k, :],
                                     start=False, stop=False)
                nc.tensor.matmul(yi, ones_row[:], wset["bi"],
                                 start=False, stop=True)
                return ps, yr, yi

            def nat(ap):
                # [p, (l h)] natural projection cols -> [p, l, h]
                return ap.rearrange("p (l h) -> p l h", h=H)

            # ---- projections, software-pipelined per tt ----
            def load_proj(p):
                wt = p if p != "o" else "q"
                ws = {c: load_kxn(w_pool, w[p, c], f"w{wt}{c}", D)
                      for c in ("r", "i", "in")}
                ws["br"] = bs[p, "r"]
                ws["bi"] = bs[p, "i"]
                return ws

            def load_x(p, tt):
                xr_t = x_pool.tile([128, KT, PT], BF16, name=f"x{p}r",
                                   tag=f"x{p}r")
                xi_t = x_pool.tile([128, KT, PT], BF16, name=f"x{p}i",
                                   tag=f"x{p}i")
                sl = bass.ts(tt, PT)
                nc.gpsimd.dma_start(
                    xr_t[:],
                    xT[f"{p}_r"][:].rearrange("(k p) n -> p k n", p=128)[:, :, sl])
                nc.gpsimd.dma_start(
                    xi_t[:],
                    xT[f"{p}_i"][:].rearrange("(k p) n -> p k n", p=128)[:, :, sl])
                return xr_t, xi_t

            prj = {p: load_proj(p) for p in ("q", "k", "v")}
            v2_of = {}
            v6_of = {}
            gn_of = {}
            hn_of = {}

            def emit_qk(p, tt):
                ws = prj[p]
                xr_t, xi_t = load_x(p, tt)
                ps, yr, yi = cproj(ws, xr_t, xi_t, tt)
                if p == "q":
                    dst = gh_pool.tile([PT, 2, H, E], BF16, name="Gn",
                                       tag="Gn")
                    gn_of[tt] = dst
                else:
                    dst = gh_pool.tile([PT, 2, H, E], BF16, name="Hn",
                                       tag="Hn")
                    hn_of[tt] = dst
                tp = evac_pool.tile([PT, D], F32, tag="tp")
                tm = evac_pool.tile([PT, D], F32, tag="tm")
                yic = evac_pool.tile([PT, D], F32, tag="yic")
                nc.vector.tensor_copy(yic[:], yi)
                nc.vector.tensor_add(tp[:], yr, yic[:])
                nc.vector.tensor_sub(tm[:], yr, yic[:])
                ty_p = 0 if p == "q" else 1
                ty_m = 1 - ty_p
                nc.gpsimd.tensor_mul(
                    dst[:, ty_p, :, :].rearrange("p h l -> p l h"),
                    nat(tp[:]), nat(tp[:]))
                nc.gpsimd.tensor_mul(
                    dst[:, ty_m, :, :].rearrange("p h l -> p l h"),
                    nat(tm[:]), nat(tm[:]))

            def emit_v(tt):
                ws = prj["v"]
                xr_t, xi_t = load_x("v", tt)
                ps, yr, yi = cproj(ws, xr_t, xi_t, tt)
                vfr = evac_pool.tile([128, H, 2, E], BF16, tag="vfr")
                vfi = evac_pool.tile([128, H, 2, E], BF16, tag="vfi")
                yr_h = yr.rearrange("p (h e) -> p h e", h=H)
                yi_h = yi.rearrange("p (h e) -> p h e", h=H)
                for dup in range(2):
                    nc.vector.tensor_copy(vfr[:, :, dup, :], yr_h)
                    nc.vector.tensor_copy(vfi[:, :, dup, :], yi_h)
                V2 = v2_pool.tile([128, H, 2, PT], BF16, tag="V2")
                nc.sync.dma_start_transpose(
                    V2[:, :, 0, :], vfr[:].rearrange("p a b c -> p (a b c)"))
                nc.sync.dma_start_transpose(
                    V2[:, :, 1, :], vfi[:].rearrange("p a b c -> p (a b c)"))
                v6 = v6_pool.tile([128, H, NJQ, NPJ, 6], BF16, tag="v6")
                v6b = v6_pool.tile([128, H, NJQ, NPJ, 6], BF16, tag="v6b")
                for c in range(2):
                    tv = V2[:, :, c, :].rearrange(
                        "p h (a blk b) -> p h a blk b", a=NJQ, blk=2)
                    nc.vector.tensor_copy(v6[0:64, :, :, :, c],
                                          tv[0:64, :, :, 0, :])
                    nc.vector.tensor_copy(v6[64:128, :, :, :, 3 + c],
                                          tv[64:128, :, :, 1, :])
                    nc.vector.tensor_scalar(v6b[0:64, :, :, :, c],
                                            tv[0:64, :, :, 0, :],
                                            INV_SQRT2, None, op0=ALU.mult)
                    nc.vector.tensor_scalar(v6b[64:128, :, :, :, 3 + c],
                                            tv[64:128, :, :, 1, :],
                                            INV_SQRT2, None, op0=ALU.mult)
                v2_of[tt] = V2
                v6_of[tt] = (v6, v6b)

            # pre-zero the staging HS buffer's zero-slots (A rows carry
            # data in blk 0 slots, B rows in blk 1; the complement stays 0)
            hs0 = stage_pool.tile([4, NPJ, H, 2, E], BF16, name="hs0",
                                  tag="HS")
            nc.vector.memset(hs0[:], 0.0)
            # preset v6 zero and ones slots on both rotating buffers
            for _ in range(2):
                for tg in ("v6", "v6b"):
                    one = 1.0 if tg == "v6" else INV_SQRT2
                    t6 = v6_pool.tile([128, H, NJQ, NPJ, 6], BF16,
                                      name=f"pre_{tg}", tag=tg)
                    nc.gpsimd.memset(t6[0:64, :, :, :, 3:6], 0.0)
                    nc.gpsimd.memset(t6[64:128, :, :, :, 0:3], 0.0)
                    nc.gpsimd.memset(t6[0:64, :, :, :, 2], one)
                    nc.gpsimd.memset(t6[64:128, :, :, :, 5], one)

            # ---- attention, with next-tt projections emitted ahead ----
            emit_qk("q", 0)
            emit_qk("k", 0)
            emit_v(0)
            for tt in range(NTT):
                V2 = v2_of.pop(tt)
                v6, v6b = v6_of.pop(tt)
                Gn = gn_of.pop(tt)
                Hn = hn_of.pop(tt)
                for jq in range(NJQ):
                    if tt + 1 < NTT:
                        if jq == 1:
                            emit_qk("q", tt + 1)
                        elif jq == 2:
                            emit_qk("k", tt + 1)
                        elif jq == 3:
                            emit_v(tt + 1)
                    arng = slice(jq * TQ, jq * TQ + NPJ)
                    brng = slice(jq * TQ + NPJ, jq * TQ + TQ)
                    GS = stage_pool.tile([4, NPJ, H, E], BF16, tag="GS")
                    HS = stage_pool.tile([4, NPJ, H, 2, E], BF16, tag="HS")
                    for r, (rng, ty) in enumerate(
                            ((arng, 0), (arng, 1), (brng, 0), (brng, 1))):
                        nc.sync.dma_start(GS[r:r + 1, :, :, :],
                                          Gn[rng, ty, :, :])
                        blk = r // 2
                        nc.sync.dma_start(HS[r:r + 1, :, :, blk, :],
                                          Hn[rng, ty, :, :])

                    ctr = ctr_pool.tile([128, KT, TQ, 3], F32, tag="ctr")
                    S = None
                    for h in range(H):
                        par = h % 2
                        k = h // 2
                        ab = ps_pool.tile([128, NPJ, E], F32, tag="ps")
                        for j in range(NPJ):
                            nc.tensor.matmul(ab[:, j, :],
                                             HS[0:4, j, h, :, :],
                                             GS[0:4, j, h, :])
                        if par == 0:
                            S = s_pool.tile([128, 2, NPJ, E], F16, tag="S")
                        nc.scalar.activation(
                            S[:, par, :, :].rearrange("p a b -> p (a b)"),
                            ab[:].rearrange("p a b -> p (a b)"),
                            AF.Sqrt, scale=SQ_SCALE)
                        if par == 0:
                            continue
                        Et1 = et_pool.tile([128, 2, NPJ, E], BF16, tag="E1")
                        Et2 = et_pool.tile([128, 2, NPJ, E], BF16, tag="E2")
                        sflat = S[:].rearrange("p a b c -> p (a b c)")
                        nc.vector.tensor_scalar(
                            Et1[:].rearrange("p a b c -> p (a b c)").bitcast(I16),
                            sflat, BPRIME, None, op0=ALU.add)
                        nc.vector.tensor_scalar(
                            Et2[:].rearrange("p a b c -> p (a b c)").bitcast(I16),
                            Et1[:].rearrange("p a b c -> p (a b c)").bitcast(I16),
                            64.0, None, op0=ALU.add)
                        for hh in (h - 1, h):
                            pp = hh % 2
                            base = 64 * pp
                            cv = ctr[base:base + 64, k, :, :].rearrange(
                                "p (blk jl) c -> p jl blk c", blk=2)
                            for j in range(NPJ):
                                dst = cv[:, j, :, :]
                                nc.tensor.matmul(
                                    dst, Et1[:, pp, j, :],
                                    v6[:, hh, jq, j, :],
                                    start=True, stop=False)
                                nc.tensor.matmul(
                                    dst, Et2[:, pp, j, :],
                                    v6b[:, hh, jq, j, :],
                                    start=False, stop=True)

                    # normalize + write o-proj operands
                    rcp = norm_pool.tile([128, KT, TQ], F32, tag="rcp")
                    nc.vector.reciprocal(rcp[:], ctr[:, :, :, 2])
                    tsl = slice(tt * PT + jq * TQ, tt * PT + (jq + 1) * TQ)
                    nc.vector.tensor_mul(VrT[:, :, tsl], ctr[:, :, :, 0],
                                         rcp[:])
                    nc.vector.tensor_mul(ViT[:, :, tsl], ctr[:, :, :, 1],
                                         rcp[:])

            # ---- phase 4: output projection ----
            wo = {c: load_kxn(w_pool, w["o", c], f"w{c}", D)
                  for c in ("r", "i", "in")}
            wo["br"] = bs["o", "r"]
            wo["bi"] = bs["o", "i"]
            for tt in range(NTT):
                ts = bass.ts(tt, PT)
                ps = ps_pool.tile([128, NPJ, E], F32, tag="ps")
                flat = ps[:].rearrange("p a b -> p (a b)")
                our = flat[:, 0:D]
                oui = flat[:, D:2 * D]
                for k in range(KT):
                    nc.tensor.matmul(our, VrT[:, k, ts], wo["r"][:, k, :],
                                     start=(k == 0), stop=False)
                for k in range(KT):
                    nc.tensor.matmul(our, ViT[:, k, ts], wo["in"][:, k, :],
                                     start=False, stop=False)
                nc.tensor.matmul(our, ones_row[:], wo["br"],
                                 start=False, stop=True)
                for k in range(KT):
                    nc.tensor.matmul(oui, ViT[:, k, ts], wo["r"][:, k, :],
                                     start=(k == 0), stop=False)
                for k in range(KT):
                    nc.tensor.matmul(oui, VrT[:, k, ts], wo["i"][:, k, :],
                                     start=False, stop=False)
                nc.tensor.matmul(oui, ones_row[:], wo["bi"],
                                 start=False, stop=True)

                sor = evac_pool.tile([PT, D], F32, tag="sor")
                soi = evac_pool.tile([PT, D], F32, tag="soi")
                nc.vector.tensor_copy(sor[:], our)
                nc.vector.tensor_copy(soi[:], oui)
                nc.sync.dma_start(out_r[ts, :], sor[:])
                nc.sync.dma_start(out_i[ts, :], soi[:])

    nc.compile()
    return nc


_NC_CACHE = None


def _get_module():
    global _NC_CACHE
    if _NC_CACHE is None:
        _NC_CACHE = _build_module()
    return _NC_CACHE


def _prep_inputs(inputs):
    """host-side shard/layout prep -> list of 8 per-core input maps"""
    import ml_dtypes
    bf = ml_dtypes.bfloat16
    TT = B * L
    xs = {nm: np.ascontiguousarray(
        np.asarray(inputs[nm]).reshape(TT, D).T.astype(bf))
        for nm in ("q_r", "q_i", "k_r", "k_i", "v_r", "v_i")}
    # h-major permutation d' = h*64 + l  ->  natural col l*H + h
    perm = np.empty(D, np.int64)
    for h in range(H):
        for l in range(E):
            perm[h * E + l] = l * H + h
    common = {}
    for p in ("q", "k", "v", "o"):
        wr = np.asarray(inputs[f"w{p}_r"]).astype(np.float32)
        wi = np.asarray(inputs[f"w{p}_i"]).astype(np.float32)
        br = np.asarray(inputs[f"b{p}_r"]).astype(np.float32)
        bi = np.asarray(inputs[f"b{p}_i"]).astype(np.float32)
        wrT = wr.T
        wiT = wi.T
        bm = br - bi
        bp = br + bi
        if p == "o":
            # o-proj contracts over h-major d': permute weight rows
            wrT = wrT[perm, :]
            wiT = wiT[perm, :]
        if p == "v":
            # v-proj emits h-major cols: permute weight cols + bias
            wrT = wrT[:, perm]
            wiT = wiT[:, perm]
            bm = bm[perm]
            bp = bp[perm]
        common[f"w_{p}_r"] = np.ascontiguousarray(wrT.astype(bf))
        common[f"w_{p}_i"] = np.ascontiguousarray(wiT.astype(bf))
        common[f"w_{p}_in"] = np.ascontiguousarray((-wiT).astype(bf))
        common[f"b_{p}_r"] = bm.reshape(1, D).astype(bf)
        common[f"b_{p}_i"] = bp.reshape(1, D).astype(bf)
    maps = []
    for c in range(NCORES):
        m = dict(common)
        sl = slice(c * T, (c + 1) * T)
        for nm, arr in xs.items():
            m[f"x_{nm}_T"] = np.ascontiguousarray(arr[:, sl])
        maps.append(m)
    return maps


def kernel(**inputs):
    nc = _get_module()
    maps = _prep_inputs(inputs)
    res = run_bass_kernel_spmd(nc, maps, core_ids=list(range(NCORES)))
    out_r = np.concatenate([res.results[c]["out_r"] for c in range(NCORES)],
                           axis=0).reshape(B, L, D)
    out_i = np.concatenate([res.results[c]["out_i"] for c in range(NCORES)],
                           axis=0).reshape(B, L, D)
    return out_r, out_i


# revision 15
# speedup vs baseline: 2.1101x; 1.0128x over previous
"""ComplexAttentionLayer Trainium2 kernel, v3 (8-core data-parallel).

Math (per token t, head h; E=64; per-head feature dim is 1, so scores are
outer products over the E axis):
  w[l,s]   = Gp[l]*Hm[s] + Gm[l]*Hp[s]       (= 2*abs2, PE outer products)
             Gp=(qr+qi)^2, Gm=(qr-qi)^2, Hp=(kr+ki)^2, Hm=(kr-ki)^2
  score    = sqrt(0.5*w)                      (ACT Sqrt table, exact)
  E        = exp(score)   via the Schraudolph bf16 bit trick on the DVE:
             bits16 = round(A2*score + 16256), A2 = 128/ln2; the bf16 with
             those bits is exp(score)*R(phi), R in [1, 1.0613] a mantissa
             sawtooth.  A second sample bits16+64 shifts the sawtooth phase
             by half a period (and multiplies by sqrt2); contracting
             E1 against V and E2 against V/sqrt2 in one accumulating PSUM
             group averages the two phases: residual error ~ +-0.8%.
  out[l]   = sum_s E[l,s] v[s] / sum_s E[l,s]  (PE per-token matmuls with a
             ones column for the denominator; DVE reciprocal+mul normalize)

The ACT engine runs ONLY the sqrt pass (one table set, loaded once); the
exp lives on the DVE at its 4x (2-byte) rate; abs2/contraction/projections
are PE matmuls; staging uses 8 flatten-DMAs per (tt, quarter) and V is
transposed with the XBAR dma_start_transpose.
"""

import math

import numpy as np

import concourse.bass as bass
import concourse.tile as tile
from concourse import bacc, mybir
from concourse.bass_utils import run_bass_kernel_spmd

AF = mybir.ActivationFunctionType
ALU = mybir.AluOpType
F32 = mybir.dt.float32
F16 = mybir.dt.float16
I16 = mybir.dt.int16
BF16 = mybir.dt.bfloat16

B, L, D, H = 4, 1024, 512, 8
E = D // H           # 64
NCORES = 8
T = B * L // NCORES  # 512 tokens per core
PT = 128             # tokens per tile
NTT = T // PT        # 4 token tiles per core
KT = D // 128        # 4 k-tiles per weight
NJQ = 4              # token quarters per tile
TQ = PT // NJQ       # 32 tokens per (tt, jq)
NPJ = TQ // 2        # 16 pairs per (tt, jq)

A2 = 128.0 / math.log(2.0)
SQ_SCALE = 0.5 * A2 * A2   # sqrt(SQ_SCALE*w) = A2*sqrt(0.5*w) = A2*score
BPRIME = 16256.0
INV_SQRT2 = 1.0 / math.sqrt(2.0)


def _build_module():
    nc = bacc.Bacc()

    xT = {}
    for nm in ("q_r", "q_i", "k_r", "k_i", "v_r", "v_i"):
        xT[nm] = nc.declare_dram_parameter(f"x_{nm}_T", [D, T], BF16, isOutput=False)
    w = {}
    for p in ("q", "k", "v", "o"):
        for c in ("r", "i", "in"):  # r = w_r.T, i = w_i.T, in = -w_i.T
            w[p, c] = nc.declare_dram_parameter(f"w_{p}_{c}", [D, D], BF16,
                                                isOutput=False)
    bias = {}
    for p in ("q", "k", "v", "o"):
        for c in ("r", "i"):  # r: br-bi, i: br+bi
            bias[p, c] = nc.declare_dram_parameter(f"b_{p}_{c}", [1, D], BF16,
                                                   isOutput=False)
    out_r = nc.declare_dram_parameter("out_r", [T, D], F32, isOutput=True)
    out_i = nc.declare_dram_parameter("out_i", [T, D], F32, isOutput=True)

    with tile.TileContext(nc) as tc:
        with (
            tc.tile_pool(name="const", bufs=1) as const_pool,
            tc.tile_pool(name="xin", bufs=1) as x_pool,
            tc.tile_pool(name="wgt", bufs=1) as w_pool,
            tc.tile_pool(name="stage", bufs=1) as stage_pool,
            tc.tile_pool(name="v2", bufs=2) as v2_pool,
            tc.tile_pool(name="v6", bufs=2) as v6_pool,
            tc.tile_pool(name="gh", bufs=2) as gh_pool,
            tc.tile_pool(name="evac", bufs=1) as evac_pool,
            tc.tile_pool(name="sco", bufs=2) as s_pool,
            tc.tile_pool(name="et", bufs=2) as et_pool,
            tc.tile_pool(name="nrm", bufs=2) as norm_pool,
            tc.tile_pool(name="ps", bufs=3, space="PSUM") as ps_pool,
            tc.tile_pool(name="psc", bufs=2, space="PSUM") as ctr_pool,
        ):
            ones_row = const_pool.tile([1, 128], BF16, tag="ones")
            nc.gpsimd.memset(ones_row[:], 1.0)

            ball = const_pool.tile([1, 8, D], BF16, tag="ball")
            bs = {}
            for i, (key, dram) in enumerate(sorted(bias.items())):
                nc.sync.dma_start(ball[:, i, :], dram[:])
                bs[key] = ball[:, i, :]


            # o-projection operands (h-major d' = h*64+l)
            VrT = const_pool.tile([128, KT, T], BF16, tag="VrT")
            ViT = const_pool.tile([128, KT, T], BF16, tag="ViT")

            def load_kxn(pool, dram, tag, n, eng=None):
                t = pool.tile([128, KT, n], BF16, name=tag, tag=tag)
                (eng or nc.gpsimd).dma_start(
                    t[:], dram[:].rearrange("(k p) n -> p k n", p=128))
                return t

            def cproj(wset, xr_t, xi_t, tt):
                """complex linear on token tile tt -> psum [128, 1024]
                (yr cols 0:512, yi cols 512:1024)"""
                ts = slice(0, PT)
                ps = ps_pool.tile([128, NPJ, E], F32, tag="ps")
                flat = ps[:].rearrange("p a b -> p (a b)")
                yr = flat[:, 0:D]
                yi = flat[:, D:2 * D]
                for k in range(KT):
                    nc.tensor.matmul(yr, xr_t[:, k, ts], wset["r"][:, k, :],
                                     start=(k == 0), stop=False)
                for k in range(KT):
                    nc.tensor.matmul(yr, xi_t[:, k, ts], wset["in"][:, k, :],
                                     start=False, stop=False)
                nc.tensor.matmul(yr, ones_row[:], wset["br"],
                                 start=False, stop=True)
                for k in range(KT):
                    nc.tensor.matmul(yi, xi_t[:, k, ts], wset["r"][:, k, :],
                                     start=(k == 0), stop=False)
                for k in range(KT):
                    nc.tensor.matmul(yi, xr_t[:, k, ts], wset["i"][:, k, :],
                                     start=False, stop=False)
                nc.tensor.matmul(yi, ones_row[:], wset["bi"],
                                 start=False, stop=True)
                return ps, yr, yi

            def nat(ap):
                # [p, (l h)] natural projection cols -> [p, l, h]
                return ap.rearrange("p (l h) -> p l h", h=H)

            # ---- projections, software-pipelined per tt ----
            def load_proj(p):
                wt = p if p != "o" else "q"
                ws = {c: load_kxn(w_pool, w[p, c], f"w{wt}{c}", D,
                                  eng=nc.sync)
                      for c in ("r", "i", "in")}
                ws["br"] = bs[p, "r"]
                ws["bi"] = bs[p, "i"]
                return ws

            def load_x(p, tt):
                xr_t = x_pool.tile([128, KT, PT], BF16, name=f"x{p}r",
                                   tag=f"x{p}r")
                xi_t = x_pool.tile([128, KT, PT], BF16, name=f"x{p}i",
                                   tag=f"x{p}i")
                sl = bass.ts(tt, PT)
                nc.gpsimd.dma_start(
                    xr_t[:],
                    xT[f"{p}_r"][:].rearrange("(k p) n -> p k n", p=128)[:, :, sl])
                nc.gpsimd.dma_start(
                    xi_t[:],
                    xT[f"{p}_i"][:].rearrange("(k p) n -> p k n", p=128)[:, :, sl])
                return xr_t, xi_t

            prj = {p: load_proj(p) for p in ("q", "k", "v")}
            v2_of = {}
            v6_of = {}
            gn_of = {}
            hn_of = {}

            def emit_qk(p, tt):
                ws = prj[p]
                xr_t, xi_t = load_x(p, tt)
                ps, yr, yi = cproj(ws, xr_t, xi_t, tt)
                if p == "q":
                    dst = gh_pool.tile([PT, 2, H, E], BF16, name="Gn",
                                       tag="Gn")
                    gn_of[tt] = dst
                else:
                    dst = gh_pool.tile([PT, 2, H, E], BF16, name="Hn",
                                       tag="Hn")
                    hn_of[tt] = dst
                tp = evac_pool.tile([PT, D], F32, tag="tp")
                tm = evac_pool.tile([PT, D], F32, tag="tm")
                yic = evac_pool.tile([PT, D], F32, tag="yic")
                nc.vector.tensor_copy(yic[:], yi)
                nc.vector.tensor_add(tp[:], yr, yic[:])
                nc.vector.tensor_sub(tm[:], yr, yic[:])
                ty_p = 0 if p == "q" else 1
                ty_m = 1 - ty_p
                nc.gpsimd.tensor_mul(
                    dst[:, ty_p, :, :].rearrange("p h l -> p l h"),
                    nat(tp[:]), nat(tp[:]))
                nc.gpsimd.tensor_mul(
                    dst[:, ty_m, :, :].rearrange("p h l -> p l h"),
                    nat(tm[:]), nat(tm[:]))

            def emit_v(tt):
                ws = prj["v"]
                xr_t, xi_t = load_x("v", tt)
                ps, yr, yi = cproj(ws, xr_t, xi_t, tt)
                vfr = evac_pool.tile([128, H, 2, E], BF16, tag="vfr")
                vfi = evac_pool.tile([128, H, 2, E], BF16, tag="vfi")
                yr_h = yr.rearrange("p (h e) -> p h e", h=H)
                yi_h = yi.rearrange("p (h e) -> p h e", h=H)
                for dup in range(2):
                    nc.vector.tensor_copy(vfr[:, :, dup, :], yr_h)
                    nc.vector.tensor_copy(vfi[:, :, dup, :], yi_h)
                V2 = v2_pool.tile([128, H, 2, PT], BF16, tag="V2")
                nc.sync.dma_start_transpose(
                    V2[:, :, 0, :], vfr[:].rearrange("p a b c -> p (a b c)"))
                nc.sync.dma_start_transpose(
                    V2[:, :, 1, :], vfi[:].rearrange("p a b c -> p (a b c)"))
                v6 = v6_pool.tile([128, H, NJQ, NPJ, 6], BF16, tag="v6")
                v6b = v6_pool.tile([128, H, NJQ, NPJ, 6], BF16, tag="v6b")
                for c in range(2):
                    tv = V2[:, :, c, :].rearrange(
                        "p h (a blk b) -> p h a blk b", a=NJQ, blk=2)
                    nc.vector.tensor_copy(v6[0:64, :, :, :, c],
                                          tv[0:64, :, :, 0, :])
                    nc.vector.tensor_copy(v6[64:128, :, :, :, 3 + c],
                                          tv[64:128, :, :, 1, :])
                    nc.vector.tensor_scalar(v6b[0:64, :, :, :, c],
                                            tv[0:64, :, :, 0, :],
                                            INV_SQRT2, None, op0=ALU.mult)
                    nc.vector.tensor_scalar(v6b[64:128, :, :, :, 3 + c],
                                            tv[64:128, :, :, 1, :],
                                            INV_SQRT2, None, op0=ALU.mult)
                v2_of[tt] = V2
                v6_of[tt] = (v6, v6b)

            # pre-zero the staging HS buffer's zero-slots (A rows carry
            # data in blk 0 slots, B rows in blk 1; the complement stays 0)
            hs0 = stage_pool.tile([4, NPJ, H, 2, E], BF16, name="hs0",
                                  tag="HS")
            nc.vector.memset(hs0[:], 0.0)
            # preset v6 zero and ones slots on both rotating buffers
            for _ in range(2):
                for tg in ("v6", "v6b"):
                    one = 1.0 if tg == "v6" else INV_SQRT2
                    t6 = v6_pool.tile([128, H, NJQ, NPJ, 6], BF16,
                                      name=f"pre_{tg}", tag=tg)
                    nc.gpsimd.memset(t6[0:64, :, :, :, 3:6], 0.0)
                    nc.gpsimd.memset(t6[64:128, :, :, :, 0:3], 0.0)
                    nc.gpsimd.memset(t6[0:64, :, :, :, 2], one)
                    nc.gpsimd.memset(t6[64:128, :, :, :, 5], one)

            # ---- attention, with next-tt projections emitted ahead ----
            emit_qk("q", 0)
            emit_qk("k", 0)
            emit_v(0)
            for tt in range(NTT):
                V2 = v2_of.pop(tt)
                v6, v6b = v6_of.pop(tt)
                Gn = gn_of.pop(tt)
                Hn = hn_of.pop(tt)
                for jq in range(NJQ):
                    if tt + 1 < NTT:
                        if jq == 1:
                            emit_qk("q", tt + 1)
                        elif jq == 2:
                            emit_qk("k", tt + 1)
                        elif jq == 3:
                            emit_v(tt + 1)
                    arng = slice(jq * TQ, jq * TQ + NPJ)
                    brng = slice(jq * TQ + NPJ, jq * TQ + TQ)
                    GS = stage_pool.tile([4, NPJ, H, E], BF16, tag="GS")
                    HS = stage_pool.tile([4, NPJ, H, 2, E], BF16, tag="HS")
                    for r, (rng, ty) in enumerate(
                            ((arng, 0), (arng, 1), (brng, 0), (brng, 1))):
                        nc.sync.dma_start(GS[r:r + 1, :, :, :],
                                          Gn[rng, ty, :, :])
                        blk = r // 2
                        nc.sync.dma_start(HS[r:r + 1, :, :, blk, :],
                                          Hn[rng, ty, :, :])

                    ctr = ctr_pool.tile([128, KT, TQ, 3], F32, tag="ctr")
                    S = None
                    for h in range(H):
                        par = h % 2
                        k = h // 2
                        ab = ps_pool.tile([128, NPJ, E], F32, tag="ps")
                        for j in range(NPJ):
                            nc.tensor.matmul(ab[:, j, :],
                                             HS[0:4, j, h, :, :],
                                             GS[0:4, j, h, :])
                        if par == 0:
                            S = s_pool.tile([128, 2, NPJ, E], F16, tag="S")
                        nc.scalar.activation(
                            S[:, par, :, :].rearrange("p a b -> p (a b)"),
                            ab[:].rearrange("p a b -> p (a b)"),
                            AF.Sqrt, scale=SQ_SCALE)
                        if par == 0:
                            continue
                        Et1 = et_pool.tile([128, 2, NPJ, E], BF16, tag="E1")
                        Et2 = et_pool.tile([128, 2, NPJ, E], BF16, tag="E2")
                        sflat = S[:].rearrange("p a b c -> p (a b c)")
                        nc.vector.tensor_scalar(
                            Et1[:].rearrange("p a b c -> p (a b c)").bitcast(I16),
                            sflat, BPRIME, None, op0=ALU.add)
                        nc.vector.tensor_scalar(
                            Et2[:].rearrange("p a b c -> p (a b c)").bitcast(I16),
                            Et1[:].rearrange("p a b c -> p (a b c)").bitcast(I16),
                            64.0, None, op0=ALU.add)
                        for hh in (h - 1, h):
                            pp = hh % 2
                            base = 64 * pp
                            cv = ctr[base:base + 64, k, :, :].rearrange(
                                "p (blk jl) c -> p jl blk c", blk=2)
                            for j in range(NPJ):
                                dst = cv[:, j, :, :]
                                nc.tensor.matmul(
                                    dst, Et1[:, pp, j, :],
                                    v6[:, hh, jq, j, :],
                                    start=True, stop=False)
                                nc.tensor.matmul(
                                    dst, Et2[:, pp, j, :],
                                    v6b[:, hh, jq, j, :],
                                    start=False, stop=True)

                    # normalize + write o-proj operands
                    rcp = norm_pool.tile([128, KT, TQ], F32, tag="rcp")
                    nc.vector.reciprocal(rcp[:], ctr[:, :, :, 2])
                    tsl = slice(tt * PT + jq * TQ, tt * PT + (jq + 1) * TQ)
                    nc.vector.tensor_mul(VrT[:, :, tsl], ctr[:, :, :, 0],
                                         rcp[:])
                    nc.vector.tensor_mul(ViT[:, :, tsl], ctr[:, :, :, 1],
                                         rcp[:])

            # ---- phase 4: output projection ----
            wo = {c: load_kxn(w_pool, w["o", c], f"w{c}", D, eng=nc.sync)
                  for c in ("r", "i", "in")}
            wo["br"] = bs["o", "r"]
            wo["bi"] = bs["o", "i"]
            for tt in range(NTT):
                ts = bass.ts(tt, PT)
                ps = ps_pool.tile([128, NPJ, E], F32, tag="ps")
                flat = ps[:].rearrange("p a b -> p (a b)")
                our = flat[:, 0:D]
                oui = flat[:, D:2 * D]
                for k in range(KT):
                    nc.tensor.matmul(our, VrT[:, k, ts], wo["r"][:, k, :],
                                     start=(k == 0), stop=False)
                for k in range(KT):
                    nc.tensor.matmul(our, ViT[:, k, ts], wo["in"][:, k, :],
                                     start=False, stop=False)
                nc.tensor.matmul(our, ones_row[:], wo["br"],
                                 start=False, stop=True)
                for k in range(KT):
                    nc.tensor.matmul(oui, ViT[:, k, ts], wo["r"][:, k, :],
                                     start=(k == 0), stop=False)
                for k in range(KT):
                    nc.tensor.matmul(oui, VrT[:, k, ts], wo["i"][:, k, :],
                                     start=False, stop=False)
                nc.tensor.matmul(oui, ones_row[:], wo["bi"],
                                 start=False, stop=True)

                sor = evac_pool.tile([PT, D], F32, tag="sor")
                soi = evac_pool.tile([PT, D], F32, tag="soi")
                nc.vector.tensor_copy(sor[:], our)
                nc.vector.tensor_copy(soi[:], oui)
                nc.sync.dma_start(out_r[ts, :], sor[:])
                nc.sync.dma_start(out_i[ts, :], soi[:])

    nc.compile()
    return nc


_NC_CACHE = None


def _get_module():
    global _NC_CACHE
    if _NC_CACHE is None:
        _NC_CACHE = _build_module()
    return _NC_CACHE


def _prep_inputs(inputs):
    """host-side shard/layout prep -> list of 8 per-core input maps"""
    import ml_dtypes
    bf = ml_dtypes.bfloat16
    TT = B * L
    xs = {nm: np.ascontiguousarray(
        np.asarray(inputs[nm]).reshape(TT, D).T.astype(bf))
        for nm in ("q_r", "q_i", "k_r", "k_i", "v_r", "v_i")}
    # h-major permutation d' = h*64 + l  ->  natural col l*H + h
    perm = np.empty(D, np.int64)
    for h in range(H):
        for l in range(E):
            perm[h * E + l] = l * H + h
    common = {}
    for p in ("q", "k", "v", "o"):
        wr = np.asarray(inputs[f"w{p}_r"]).astype(np.float32)
        wi = np.asarray(inputs[f"w{p}_i"]).astype(np.float32)
        br = np.asarray(inputs[f"b{p}_r"]).astype(np.float32)
        bi = np.asarray(inputs[f"b{p}_i"]).astype(np.float32)
        wrT = wr.T
        wiT = wi.T
        bm = br - bi
        bp = br + bi
        if p == "o":
            # o-proj contracts over h-major d': permute weight rows
            wrT = wrT[perm, :]
            wiT = wiT[perm, :]
        if p == "v":
            # v-proj emits h-major cols: permute weight cols + bias
            wrT = wrT[:, perm]
            wiT = wiT[:, perm]
            bm = bm[perm]
            bp = bp[perm]
        common[f"w_{p}_r"] = np.ascontiguousarray(wrT.astype(bf))
        common[f"w_{p}_i"] = np.ascontiguousarray(wiT.astype(bf))
        common[f"w_{p}_in"] = np.ascontiguousarray((-wiT).astype(bf))
        common[f"b_{p}_r"] = bm.reshape(1, D).astype(bf)
        common[f"b_{p}_i"] = bp.reshape(1, D).astype(bf)
    maps = []
    for c in range(NCORES):
        m = dict(common)
        sl = slice(c * T, (c + 1) * T)
        for nm, arr in xs.items():
            m[f"x_{nm}_T"] = np.ascontiguousarray(arr[:, sl])
        maps.append(m)
    return maps


def kernel(**inputs):
    nc = _get_module()
    maps = _prep_inputs(inputs)
    res = run_bass_kernel_spmd(nc, maps, core_ids=list(range(NCORES)))
    out_r = np.concatenate([res.results[c]["out_r"] for c in range(NCORES)],
                           axis=0).reshape(B, L, D)
    out_i = np.concatenate([res.results[c]["out_i"] for c in range(NCORES)],
                           axis=0).reshape(B, L, D)
    return out_r, out_i


# revision 26
# speedup vs baseline: 2.4610x; 1.1663x over previous
"""ComplexAttentionLayer Trainium2 kernel, v3 (8-core data-parallel).

Math (per token t, head h; E=64; per-head feature dim is 1, so scores are
outer products over the E axis):
  w[l,s]   = Gp[l]*Hm[s] + Gm[l]*Hp[s]       (= 2*abs2, PE outer products)
             Gp=(qr+qi)^2, Gm=(qr-qi)^2, Hp=(kr+ki)^2, Hm=(kr-ki)^2
  score    = sqrt(0.5*w)                      (ACT Sqrt table, exact)
  E        = exp(score)   via the Schraudolph bf16 bit trick on the DVE:
             bits16 = round(A2*score + 16256), A2 = 128/ln2; the bf16 with
             those bits is exp(score)*R(phi), R in [1, 1.0613] a mantissa
             sawtooth.  A second sample bits16+64 shifts the sawtooth phase
             by half a period (and multiplies by sqrt2); contracting
             E1 against V and E2 against V/sqrt2 in one accumulating PSUM
             group averages the two phases: residual error ~ +-0.8%.
  out[l]   = sum_s E[l,s] v[s] / sum_s E[l,s]  (PE per-token matmuls with a
             ones column for the denominator; DVE reciprocal+mul normalize)

The ACT engine runs ONLY the sqrt pass (one table set, loaded once); the
exp lives on the DVE at its 4x (2-byte) rate; abs2/contraction/projections
are PE matmuls; staging uses 8 flatten-DMAs per (tt, quarter) and V is
transposed with the XBAR dma_start_transpose.
"""

import math

import numpy as np

import concourse.bass as bass
import concourse.tile as tile
from concourse import bacc, mybir
from concourse.bass_utils import run_bass_kernel_spmd

AF = mybir.ActivationFunctionType
ALU = mybir.AluOpType
F32 = mybir.dt.float32
F16 = mybir.dt.float16
I16 = mybir.dt.int16
BF16 = mybir.dt.bfloat16

B, L, D, H = 4, 1024, 512, 8
E = D // H           # 64
NCORES = 8
T = B * L // NCORES  # 512 tokens per core
PT = 128             # tokens per tile
NTT = T // PT        # 4 token tiles per core
KT = D // 128        # 4 k-tiles per weight
NJQ = 4              # token quarters per tile
TQ = PT // NJQ       # 32 tokens per (tt, jq)
NPJ = TQ // 2        # 16 pairs per (tt, jq)

A2 = 128.0 / math.log(2.0)
SQ_SCALE = 0.5 * A2 * A2   # sqrt(SQ_SCALE*w) = A2*sqrt(0.5*w) = A2*score
BPRIME = 16256.0
INV_SQRT2 = 1.0 / math.sqrt(2.0)


def _build_module():
    nc = bacc.Bacc()

    xT = {}
    for nm in ("q_r", "q_i", "k_r", "k_i", "v_r", "v_i"):
        xT[nm] = nc.declare_dram_parameter(f"x_{nm}_T", [D, T], BF16, isOutput=False)
    w = {}
    for p in ("q", "k", "v", "o"):
        for c in ("r", "i", "in"):  # r = w_r.T, i = w_i.T, in = -w_i.T
            w[p, c] = nc.declare_dram_parameter(f"w_{p}_{c}", [D, D], BF16,
                                                isOutput=False)
    bias = {}
    for p in ("q", "k", "v", "o"):
        for c in ("r", "i"):  # r: br-bi, i: br+bi
            bias[p, c] = nc.declare_dram_parameter(f"b_{p}_{c}", [1, D], BF16,
                                                   isOutput=False)
    out_r = nc.declare_dram_parameter("out_r", [T, D], F32, isOutput=True)
    out_i = nc.declare_dram_parameter("out_i", [T, D], F32, isOutput=True)

    with tile.TileContext(nc) as tc:
        with (
            tc.tile_pool(name="const", bufs=1) as const_pool,
            tc.tile_pool(name="xin", bufs=1) as x_pool,
            tc.tile_pool(name="wgt", bufs=1) as w_pool,
            tc.tile_pool(name="stage", bufs=1) as stage_pool,
            tc.tile_pool(name="gstage", bufs=2) as gs_pool,
            tc.tile_pool(name="v2", bufs=1) as v2_pool,
            tc.tile_pool(name="v6", bufs=2) as v6_pool,
            tc.tile_pool(name="gh", bufs=2) as gh_pool,
            tc.tile_pool(name="evac", bufs=1) as evac_pool,
            tc.tile_pool(name="sco", bufs=2) as s_pool,
            tc.tile_pool(name="et", bufs=1) as et_pool,
            tc.tile_pool(name="nrm", bufs=1) as norm_pool,
            tc.tile_pool(name="ps", bufs=3, space="PSUM") as ps_pool,
            tc.tile_pool(name="psc", bufs=2, space="PSUM") as ctr_pool,
        ):
            ones_row = const_pool.tile([1, 128], BF16, tag="ones")
            nc.gpsimd.memset(ones_row[:], 1.0)

            ball = const_pool.tile([1, 8, D], BF16, tag="ball")
            bs = {}


            # o-projection operands (h-major d' = h*64+l)
            VrT = const_pool.tile([128, KT, T], BF16, tag="VrT")
            ViT = const_pool.tile([128, KT, T], BF16, tag="ViT")

            def load_kxn(pool, dram, tag, n, eng=None):
                t = pool.tile([128, KT, n], BF16, name=tag, tag=tag)
                (eng or nc.gpsimd).dma_start(
                    t[:], dram[:].rearrange("(k p) n -> p k n", p=128))
                return t

            def cproj(wset, xr_t, xi_t, tt):
                """complex linear on token tile tt -> psum [128, 1024]
                (yr cols 0:512, yi cols 512:1024)"""
                ts = slice(0, PT)
                ps = ps_pool.tile([128, NPJ, E], F32, tag="ps")
                flat = ps[:].rearrange("p a b -> p (a b)")
                yr = flat[:, 0:D]
                yi = flat[:, D:2 * D]
                for k in range(KT):
                    nc.tensor.matmul(yr, xr_t[:, k, ts], wset["r"][:, k, :],
                                     start=(k == 0), stop=False)
                for k in range(KT):
                    nc.tensor.matmul(yr, xi_t[:, k, ts], wset["in"][:, k, :],
                                     start=False, stop=False)
                nc.tensor.matmul(yr, ones_row[:], wset["br"],
                                 start=False, stop=True)
                for k in range(KT):
                    nc.tensor.matmul(yi, xi_t[:, k, ts], wset["r"][:, k, :],
                                     start=(k == 0), stop=False)
                for k in range(KT):
                    nc.tensor.matmul(yi, xr_t[:, k, ts], wset["i"][:, k, :],
                                     start=False, stop=False)
                nc.tensor.matmul(yi, ones_row[:], wset["bi"],
                                 start=False, stop=True)
                return ps, yr, yi

            def nat(ap):
                # [p, (l h)] natural projection cols -> [p, l, h]
                return ap.rearrange("p (l h) -> p l h", h=H)

            # ---- projections, software-pipelined per tt ----
            def load_proj(p):
                wt = p if p != "o" else "q"
                ws = {c: load_kxn(w_pool, w[p, c], f"w{wt}{c}", D,
                                  eng=nc.sync)
                      for c in ("r", "i", "in")}
                ws["br"] = bs[p, "r"]
                ws["bi"] = bs[p, "i"]
                return ws

            def load_biases():
                for i, (key, dram) in enumerate(sorted(bias.items())):
                    nc.sync.dma_start(ball[:, i, :], dram[:])
                    bs[key] = ball[:, i, :]

            def load_x(p, tt):
                xr_t = x_pool.tile([128, KT, PT], BF16, name=f"x{p}r",
                                   tag=f"x{p}r")
                xi_t = x_pool.tile([128, KT, PT], BF16, name=f"x{p}i",
                                   tag=f"x{p}i")
                sl = bass.ts(tt, PT)
                nc.gpsimd.dma_start(
                    xr_t[:],
                    xT[f"{p}_r"][:].rearrange("(k p) n -> p k n", p=128)[:, :, sl])
                nc.gpsimd.dma_start(
                    xi_t[:],
                    xT[f"{p}_i"][:].rearrange("(k p) n -> p k n", p=128)[:, :, sl])
                return xr_t, xi_t

            # PE p-state warmup: ~3us of junk matmuls while inputs load
            for _ in range(2):
                wps = ps_pool.tile([128, NPJ, E], F32, tag="ps")
                wf = wps[:].rearrange("p a b -> p (a b)")
                for i in range(8):
                    nc.tensor.matmul(wf[:, bass.ts(i, 128)], ones_row[:],
                                     ones_row[:])

            prj = {}
            for pp_ in ("q", "k", "v"):
                prj[pp_] = None  # placeholder, filled below in load order
            # q weights first (they gate the first projection), then biases,
            # then the rest
            ws_q = {c: load_kxn(w_pool, w["q", c], f"wq{c}", D, eng=nc.sync)
                    for c in ("r", "i", "in")}
            load_biases()
            ws_q["br"] = bs["q", "r"]
            ws_q["bi"] = bs["q", "i"]
            prj["q"] = ws_q
            for pp_ in ("k", "v"):
                prj[pp_] = load_proj(pp_)
            v2_of = {}
            v6_of = {}
            gn_of = {}
            hn_of = {}

            def emit_qk(p, tt):
                ws = prj[p]
                xr_t, xi_t = load_x(p, tt)
                ps, yr, yi = cproj(ws, xr_t, xi_t, tt)
                if p == "q":
                    dst = gh_pool.tile([PT, 2, H, E], BF16, name="Gn",
                                       tag="Gn")
                    gn_of[tt] = dst
                else:
                    dst = gh_pool.tile([PT, 2, H, E], BF16, name="Hn",
                                       tag="Hn")
                    hn_of[tt] = dst
                tp = evac_pool.tile([PT, D], F32, tag="tp")
                tm = evac_pool.tile([PT, D], F32, tag="tm")
                yic = evac_pool.tile([PT, D], F32, tag="yic")
                nc.vector.tensor_copy(yic[:], yi)
                nc.vector.tensor_add(tp[:], yr, yic[:])
                nc.vector.tensor_sub(tm[:], yr, yic[:])
                ty_p = 0 if p == "q" else 1
                ty_m = 1 - ty_p
                nc.gpsimd.tensor_mul(
                    dst[:, ty_p, :, :].rearrange("p h l -> p l h"),
                    nat(tp[:]), nat(tp[:]))
                nc.gpsimd.tensor_mul(
                    dst[:, ty_m, :, :].rearrange("p h l -> p l h"),
                    nat(tm[:]), nat(tm[:]))

            def emit_v(tt):
                ws = prj["v"]
                xr_t, xi_t = load_x("v", tt)
                ps, yr, yi = cproj(ws, xr_t, xi_t, tt)
                vfr = evac_pool.tile([128, H, 2, E], BF16, tag="vfr")
                vfi = evac_pool.tile([128, H, 2, E], BF16, tag="vfi")
                yr_h = yr.rearrange("p (h e) -> p h e", h=H)
                yi_h = yi.rearrange("p (h e) -> p h e", h=H)
                for dup in range(2):
                    nc.scalar.copy(vfr[:, :, dup, :], yr_h)
                    nc.scalar.copy(vfi[:, :, dup, :], yi_h)
                V2 = v2_pool.tile([128, H, 2, PT], BF16, tag="V2")
                nc.sync.dma_start_transpose(
                    V2[:, :, 0, :], vfr[:].rearrange("p a b c -> p (a b c)"))
                nc.sync.dma_start_transpose(
                    V2[:, :, 1, :], vfi[:].rearrange("p a b c -> p (a b c)"))
                v6 = v6_pool.tile([128, H, NJQ, NPJ, 6], BF16, tag="v6")
                v6b = v6_pool.tile([128, H, NJQ, NPJ, 6], BF16, tag="v6b")
                for c in range(2):
                    tv = V2[:, :, c, :].rearrange(
                        "p h (a blk b) -> p h a blk b", a=NJQ, blk=2)
                    nc.vector.tensor_copy(v6[0:64, :, :, :, c],
                                          tv[0:64, :, :, 0, :])
                    nc.vector.tensor_copy(v6[64:128, :, :, :, 3 + c],
                                          tv[64:128, :, :, 1, :])
                    nc.vector.tensor_scalar(v6b[0:64, :, :, :, c],
                                            tv[0:64, :, :, 0, :],
                                            INV_SQRT2, None, op0=ALU.mult)
                    nc.vector.tensor_scalar(v6b[64:128, :, :, :, 3 + c],
                                            tv[64:128, :, :, 1, :],
                                            INV_SQRT2, None, op0=ALU.mult)
                v2_of[tt] = V2
                v6_of[tt] = (v6, v6b)

            # pre-zero the staging HS buffer's zero-slots (A rows carry
            # data in blk 0 slots, B rows in blk 1; the complement stays 0)
            hs0 = stage_pool.tile([4, NPJ, H, 2, E], BF16, name="hs0",
                                  tag="HS")
            nc.vector.memset(hs0[:], 0.0)
            # preset v6 zero and ones slots on both rotating buffers
            for _ in range(2):
                for tg in ("v6", "v6b"):
                    one = 1.0 if tg == "v6" else INV_SQRT2
                    t6 = v6_pool.tile([128, H, NJQ, NPJ, 6], BF16,
                                      name=f"pre_{tg}", tag=tg)
                    nc.vector.memset(t6[0:64, :, :, :, 3:6], 0.0)
                    nc.vector.memset(t6[64:128, :, :, :, 0:3], 0.0)
                    nc.vector.memset(t6[0:64, :, :, :, 2], one)
                    nc.vector.memset(t6[64:128, :, :, :, 5], one)

            # ---- attention, with next-tt projections emitted ahead ----
            emit_qk("q", 0)
            emit_qk("k", 0)
            emit_v(0)
            def emit_staging(stt, sjq):
                arng = slice(sjq * TQ, sjq * TQ + NPJ)
                brng = slice(sjq * TQ + NPJ, sjq * TQ + TQ)
                Gn = gn_of[stt]
                Hn = hn_of[stt]
                GS = gs_pool.tile([4, NPJ, H, E], BF16, name="GS", tag="GS")
                HS = stage_pool.tile([4, NPJ, H, 2, E], BF16, name="HS",
                                     tag="HS")
                for r, (rng, ty) in enumerate(
                        ((arng, 0), (arng, 1), (brng, 0), (brng, 1))):
                    nc.sync.dma_start(GS[r:r + 1, :, :, :],
                                      Gn[rng, ty, :, :])
                    blk = r // 2
                    nc.sync.dma_start(HS[r:r + 1, :, :, blk, :],
                                      Hn[rng, ty, :, :])
                st_of[(stt, sjq)] = (GS, HS)

            st_of = {}
            emit_staging(0, 0)
            for tt in range(NTT):
                V2 = v2_of.pop(tt)
                v6, v6b = v6_of.pop(tt)
                for jq in range(NJQ):
                    # issue next quarter's staging ahead of everything else
                    njq = (jq + 1) % NJQ
                    ntt = tt + (1 if njq == 0 else 0)
                    if ntt < NTT and (tt + 1 < NTT or njq != 0):
                        if (ntt, njq) == (tt + 1, 0):
                            pass  # deferred below until Gn/Hn exist
                        else:
                            emit_staging(ntt, njq)
                    if tt + 1 < NTT:
                        if jq == 1:
                            emit_qk("q", tt + 1)
                        elif jq == 2:
                            emit_qk("k", tt + 1)
                            emit_staging(tt + 1, 0)
                        elif jq == 3:
                            emit_v(tt + 1)
                    GS, HS = st_of.pop((tt, jq))

                    ctr = ctr_pool.tile([128, KT, TQ, 3], F32, tag="ctr")
                    S = None
                    for h in range(H):
                        par = h % 2
                        k = h // 2
                        ab = ps_pool.tile([128, NPJ, E], F32, tag="ps")
                        for j in range(NPJ):
                            nc.tensor.matmul(ab[:, j, :],
                                             HS[0:4, j, h, :, :],
                                             GS[0:4, j, h, :])
                        if par == 0:
                            S = s_pool.tile([128, 2, NPJ, E], F16, tag="S")
                        nc.scalar.activation(
                            S[:, par, :, :].rearrange("p a b -> p (a b)"),
                            ab[:].rearrange("p a b -> p (a b)"),
                            AF.Sqrt, scale=SQ_SCALE)
                        if par == 0:
                            continue
                        Et1 = et_pool.tile([128, 2, NPJ, E], BF16, tag="E1")
                        Et2 = et_pool.tile([128, 2, NPJ, E], BF16, tag="E2")
                        sflat = S[:].rearrange("p a b c -> p (a b c)")
                        nc.vector.tensor_scalar(
                            Et1[:].rearrange("p a b c -> p (a b c)").bitcast(I16),
                            sflat, BPRIME, None, op0=ALU.add)
                        nc.vector.tensor_scalar(
                            Et2[:].rearrange("p a b c -> p (a b c)").bitcast(I16),
                            Et1[:].rearrange("p a b c -> p (a b c)").bitcast(I16),
                            64.0, None, op0=ALU.add)
                        for hh in (h - 1, h):
                            pp = hh % 2
                            base = 64 * pp
                            cv = ctr[base:base + 64, k, :, :].rearrange(
                                "p (blk jl) c -> p jl blk c", blk=2)
                            for j in range(NPJ):
                                dst = cv[:, j, :, :]
                                nc.tensor.matmul(
                                    dst, Et1[:, pp, j, :],
                                    v6[:, hh, jq, j, :],
                                    start=True, stop=False)
                                nc.tensor.matmul(
                                    dst, Et2[:, pp, j, :],
                                    v6b[:, hh, jq, j, :],
                                    start=False, stop=True)

                    # normalize + write o-proj operands
                    rcp = norm_pool.tile([128, KT, TQ], F32, tag="rcp")
                    nc.vector.reciprocal(rcp[:], ctr[:, :, :, 2])
                    tsl = slice(tt * PT + jq * TQ, tt * PT + (jq + 1) * TQ)
                    nc.vector.tensor_mul(VrT[:, :, tsl], ctr[:, :, :, 0],
                                         rcp[:])
                    nc.vector.tensor_mul(ViT[:, :, tsl], ctr[:, :, :, 1],
                                         rcp[:])

            # ---- phase 4: output projection ----
            wo = {c: load_kxn(w_pool, w["o", c], f"w{c}", D, eng=nc.sync)
                  for c in ("r", "i", "in")}
            wo["br"] = bs["o", "r"]
            wo["bi"] = bs["o", "i"]
            for tt in range(NTT):
                ts = bass.ts(tt, PT)
                ps = ps_pool.tile([128, NPJ, E], F32, tag="ps")
                flat = ps[:].rearrange("p a b -> p (a b)")
                our = flat[:, 0:D]
                oui = flat[:, D:2 * D]
                for k in range(KT):
                    nc.tensor.matmul(our, VrT[:, k, ts], wo["r"][:, k, :],
                                     start=(k == 0), stop=False)
                for k in range(KT):
                    nc.tensor.matmul(our, ViT[:, k, ts], wo["in"][:, k, :],
                                     start=False, stop=False)
                nc.tensor.matmul(our, ones_row[:], wo["br"],
                                 start=False, stop=True)
                for k in range(KT):
                    nc.tensor.matmul(oui, ViT[:, k, ts], wo["r"][:, k, :],
                                     start=(k == 0), stop=False)
                for k in range(KT):
                    nc.tensor.matmul(oui, VrT[:, k, ts], wo["i"][:, k, :],
                                     start=False, stop=False)
                nc.tensor.matmul(oui, ones_row[:], wo["bi"],
                                 start=False, stop=True)

                sor = evac_pool.tile([PT, D], F32, tag="sor")
                soi = evac_pool.tile([PT, D], F32, tag="soi")
                nc.scalar.copy(sor[:], our)
                nc.scalar.copy(soi[:], oui)
                nc.sync.dma_start(out_r[ts, :], sor[:])
                nc.sync.dma_start(out_i[ts, :], soi[:])

    nc.compile()
    return nc


_NC_CACHE = None


def _get_module():
    global _NC_CACHE
    if _NC_CACHE is None:
        _NC_CACHE = _build_module()
    return _NC_CACHE


def _prep_inputs(inputs):
    """host-side shard/layout prep -> list of 8 per-core input maps"""
    import ml_dtypes
    bf = ml_dtypes.bfloat16
    TT = B * L
    xs = {nm: np.ascontiguousarray(
        np.asarray(inputs[nm]).reshape(TT, D).T.astype(bf))
        for nm in ("q_r", "q_i", "k_r", "k_i", "v_r", "v_i")}
    # h-major permutation d' = h*64 + l  ->  natural col l*H + h
    perm = np.empty(D, np.int64)
    for h in range(H):
        for l in range(E):
            perm[h * E + l] = l * H + h
    common = {}
    for p in ("q", "k", "v", "o"):
        wr = np.asarray(inputs[f"w{p}_r"]).astype(np.float32)
        wi = np.asarray(inputs[f"w{p}_i"]).astype(np.float32)
        br = np.asarray(inputs[f"b{p}_r"]).astype(np.float32)
        bi = np.asarray(inputs[f"b{p}_i"]).astype(np.float32)
        wrT = wr.T
        wiT = wi.T
        bm = br - bi
        bp = br + bi
        if p == "o":
            # o-proj contracts over h-major d': permute weight rows
            wrT = wrT[perm, :]
            wiT = wiT[perm, :]
        if p == "v":
            # v-proj emits h-major cols: permute weight cols + bias
            wrT = wrT[:, perm]
            wiT = wiT[:, perm]
            bm = bm[perm]
            bp = bp[perm]
        common[f"w_{p}_r"] = np.ascontiguousarray(wrT.astype(bf))
        common[f"w_{p}_i"] = np.ascontiguousarray(wiT.astype(bf))
        common[f"w_{p}_in"] = np.ascontiguousarray((-wiT).astype(bf))
        common[f"b_{p}_r"] = bm.reshape(1, D).astype(bf)
        common[f"b_{p}_i"] = bp.reshape(1, D).astype(bf)
    maps = []
    for c in range(NCORES):
        m = dict(common)
        sl = slice(c * T, (c + 1) * T)
        for nm, arr in xs.items():
            m[f"x_{nm}_T"] = np.ascontiguousarray(arr[:, sl])
        maps.append(m)
    return maps


def kernel(**inputs):
    nc = _get_module()
    maps = _prep_inputs(inputs)
    res = run_bass_kernel_spmd(nc, maps, core_ids=list(range(NCORES)))
    out_r = np.concatenate([res.results[c]["out_r"] for c in range(NCORES)],
                           axis=0).reshape(B, L, D)
    out_i = np.concatenate([res.results[c]["out_i"] for c in range(NCORES)],
                           axis=0).reshape(B, L, D)
    return out_r, out_i


# revision 30
# speedup vs baseline: 2.5157x; 1.0223x over previous
"""ComplexAttentionLayer Trainium2 kernel, v3 (8-core data-parallel).

Math (per token t, head h; E=64; per-head feature dim is 1, so scores are
outer products over the E axis):
  w[l,s]   = Gp[l]*Hm[s] + Gm[l]*Hp[s]       (= 2*abs2, PE outer products)
             Gp=(qr+qi)^2, Gm=(qr-qi)^2, Hp=(kr+ki)^2, Hm=(kr-ki)^2
  score    = sqrt(0.5*w)                      (ACT Sqrt table, exact)
  E        = exp(score)   via the Schraudolph bf16 bit trick on the DVE:
             bits16 = round(A2*score + 16256), A2 = 128/ln2; the bf16 with
             those bits is exp(score)*R(phi), R in [1, 1.0613] a mantissa
             sawtooth.  A second sample bits16+64 shifts the sawtooth phase
             by half a period (and multiplies by sqrt2); contracting
             E1 against V and E2 against V/sqrt2 in one accumulating PSUM
             group averages the two phases: residual error ~ +-0.8%.
  out[l]   = sum_s E[l,s] v[s] / sum_s E[l,s]  (PE per-token matmuls with a
             ones column for the denominator; DVE reciprocal+mul normalize)

The ACT engine runs ONLY the sqrt pass (one table set, loaded once); the
exp lives on the DVE at its 4x (2-byte) rate; abs2/contraction/projections
are PE matmuls; staging uses 8 flatten-DMAs per (tt, quarter) and V is
transposed with the XBAR dma_start_transpose.
"""

import math

import numpy as np

import concourse.bass as bass
import concourse.tile as tile
from concourse import bacc, mybir
from concourse.bass_utils import run_bass_kernel_spmd

AF = mybir.ActivationFunctionType
ALU = mybir.AluOpType
F32 = mybir.dt.float32
F16 = mybir.dt.float16
I16 = mybir.dt.int16
BF16 = mybir.dt.bfloat16

B, L, D, H = 4, 1024, 512, 8
E = D // H           # 64
NCORES = 8
T = B * L // NCORES  # 512 tokens per core
PT = 128             # tokens per tile
NTT = T // PT        # 4 token tiles per core
KT = D // 128        # 4 k-tiles per weight
NJQ = 4              # token quarters per tile
TQ = PT // NJQ       # 32 tokens per (tt, jq)
NPJ = TQ // 2        # 16 pairs per (tt, jq)

A2 = 128.0 / math.log(2.0)
SQ_SCALE = 0.5 * A2 * A2   # sqrt(SQ_SCALE*w) = A2*sqrt(0.5*w) = A2*score
BPRIME = 16256.0
INV_SQRT2 = 1.0 / math.sqrt(2.0)


def _build_module():
    nc = bacc.Bacc()

    xT = {}
    for nm in ("q_r", "q_i", "k_r", "k_i", "v_r", "v_i"):
        xT[nm] = nc.declare_dram_parameter(f"x_{nm}_T", [D, T], BF16, isOutput=False)
    w = {}
    for p in ("q", "k", "v", "o"):
        for c in ("r", "i", "in"):  # r = w_r.T, i = w_i.T, in = -w_i.T
            w[p, c] = nc.declare_dram_parameter(f"w_{p}_{c}", [D, D], BF16,
                                                isOutput=False)
    bias = {}
    for p in ("q", "k", "v", "o"):
        for c in ("r", "i"):  # r: br-bi, i: br+bi
            bias[p, c] = nc.declare_dram_parameter(f"b_{p}_{c}", [1, D], BF16,
                                                   isOutput=False)
    out_r = nc.declare_dram_parameter("out_r", [T, D], F32, isOutput=True)
    out_i = nc.declare_dram_parameter("out_i", [T, D], F32, isOutput=True)

    with tile.TileContext(nc) as tc:
        with (
            tc.tile_pool(name="const", bufs=1) as const_pool,
            tc.tile_pool(name="xin", bufs=1) as x_pool,
            tc.tile_pool(name="wgt", bufs=1) as w_pool,
            tc.tile_pool(name="stage", bufs=1) as stage_pool,
            tc.tile_pool(name="gstage", bufs=2) as gs_pool,
            tc.tile_pool(name="v2", bufs=1) as v2_pool,
            tc.tile_pool(name="v6", bufs=2) as v6_pool,
            tc.tile_pool(name="gh", bufs=2) as gh_pool,
            tc.tile_pool(name="evac", bufs=1) as evac_pool,
            tc.tile_pool(name="sco", bufs=2) as s_pool,
            tc.tile_pool(name="et", bufs=1) as et_pool,
            tc.tile_pool(name="nrm", bufs=1) as norm_pool,
            tc.tile_pool(name="ps", bufs=3, space="PSUM") as ps_pool,
            tc.tile_pool(name="psc", bufs=2, space="PSUM") as ctr_pool,
        ):
            ones_row = const_pool.tile([1, 128], BF16, tag="ones")
            nc.gpsimd.memset(ones_row[:], 1.0)

            ball = const_pool.tile([1, 8, D], BF16, tag="ball")
            bs = {}


            # o-projection operands (h-major d' = h*64+l)
            VrT = const_pool.tile([128, KT, T], BF16, tag="VrT")
            ViT = const_pool.tile([128, KT, T], BF16, tag="ViT")

            def load_kxn(pool, dram, tag, n, eng=None):
                t = pool.tile([128, KT, n], BF16, name=tag, tag=tag)
                (eng or nc.gpsimd).dma_start(
                    t[:], dram[:].rearrange("(k p) n -> p k n", p=128))
                return t

            def cproj(wset, xr_t, xi_t, tt):
                """complex linear on token tile tt -> psum [128, 1024]
                (yr cols 0:512, yi cols 512:1024)"""
                ts = slice(0, PT)
                ps = ps_pool.tile([128, NPJ, E], F32, tag="ps")
                flat = ps[:].rearrange("p a b -> p (a b)")
                yr = flat[:, 0:D]
                yi = flat[:, D:2 * D]
                for k in range(KT):
                    nc.tensor.matmul(yr, xr_t[:, k, ts], wset["r"][:, k, :],
                                     start=(k == 0), stop=False)
                for k in range(KT):
                    nc.tensor.matmul(yr, xi_t[:, k, ts], wset["in"][:, k, :],
                                     start=False, stop=False)
                nc.tensor.matmul(yr, ones_row[:], wset["br"],
                                 start=False, stop=True)
                for k in range(KT):
                    nc.tensor.matmul(yi, xi_t[:, k, ts], wset["r"][:, k, :],
                                     start=(k == 0), stop=False)
                for k in range(KT):
                    nc.tensor.matmul(yi, xr_t[:, k, ts], wset["i"][:, k, :],
                                     start=False, stop=False)
                nc.tensor.matmul(yi, ones_row[:], wset["bi"],
                                 start=False, stop=True)
                return ps, yr, yi

            def nat(ap):
                # [p, (l h)] natural projection cols -> [p, l, h]
                return ap.rearrange("p (l h) -> p l h", h=H)

            # ---- projections, software-pipelined per tt ----
            def load_proj(p):
                wt = p if p != "o" else "q"
                ws = {c: load_kxn(w_pool, w[p, c], f"w{wt}{c}", D,
                                  eng=nc.sync)
                      for c in ("r", "i", "in")}
                ws["br"] = bs[p, "r"]
                ws["bi"] = bs[p, "i"]
                return ws

            def load_biases():
                for i, (key, dram) in enumerate(sorted(bias.items())):
                    nc.sync.dma_start(ball[:, i, :], dram[:])
                    bs[key] = ball[:, i, :]

            def load_x(p, tt):
                xr_t = x_pool.tile([128, KT, PT], BF16, name=f"x{p}r",
                                   tag=f"x{p}r")
                xi_t = x_pool.tile([128, KT, PT], BF16, name=f"x{p}i",
                                   tag=f"x{p}i")
                sl = bass.ts(tt, PT)
                nc.gpsimd.dma_start(
                    xr_t[:],
                    xT[f"{p}_r"][:].rearrange("(k p) n -> p k n", p=128)[:, :, sl])
                nc.gpsimd.dma_start(
                    xi_t[:],
                    xT[f"{p}_i"][:].rearrange("(k p) n -> p k n", p=128)[:, :, sl])
                return xr_t, xi_t

            # PE p-state warmup: ~3us of junk matmuls while inputs load
            for _ in range(2):
                wps = ps_pool.tile([128, NPJ, E], F32, tag="ps")
                wf = wps[:].rearrange("p a b -> p (a b)")
                for i in range(8):
                    nc.tensor.matmul(wf[:, bass.ts(i, 128)], ones_row[:],
                                     ones_row[:])

            prj = {}
            for pp_ in ("q", "k", "v"):
                prj[pp_] = None  # placeholder, filled below in load order
            # q weights first (they gate the first projection), then biases,
            # then the rest
            ws_q = {c: load_kxn(w_pool, w["q", c], f"wq{c}", D, eng=nc.sync)
                    for c in ("r", "i", "in")}
            load_biases()
            ws_q["br"] = bs["q", "r"]
            ws_q["bi"] = bs["q", "i"]
            prj["q"] = ws_q
            for pp_ in ("k", "v"):
                prj[pp_] = load_proj(pp_)
            v2_of = {}
            v6_of = {}
            gn_of = {}
            hn_of = {}

            def emit_qk(p, tt):
                # host-prepped weight combos make the projection emit
                # yr+yi (yr slot) and -(yr-yi) (yi slot) directly; one ACT
                # Square per half evacuates PSUM into Gn/Hn (sign drops)
                ws = prj[p]
                xr_t, xi_t = load_x(p, tt)
                ps, yr, yi = cproj(ws, xr_t, xi_t, tt)
                if p == "q":
                    dst = gh_pool.tile([PT, 2, H, E], BF16, name="Gn",
                                       tag="Gn")
                    gn_of[tt] = dst
                else:
                    dst = gh_pool.tile([PT, 2, H, E], BF16, name="Hn",
                                       tag="Hn")
                    hn_of[tt] = dst
                ty_p = 0 if p == "q" else 1
                ty_m = 1 - ty_p
                nc.scalar.activation(
                    dst[:, ty_p, :, :].rearrange("p h l -> p l h"),
                    nat(yr), AF.Square)
                nc.scalar.activation(
                    dst[:, ty_m, :, :].rearrange("p h l -> p l h"),
                    nat(yi), AF.Square)

            def emit_v(tt):
                ws = prj["v"]
                xr_t, xi_t = load_x("v", tt)
                ps, yr, yi = cproj(ws, xr_t, xi_t, tt)
                vfr = evac_pool.tile([128, H, 2, E], BF16, tag="vfr")
                vfi = evac_pool.tile([128, H, 2, E], BF16, tag="vfi")
                yr_h = yr.rearrange("p (h e) -> p h e", h=H)
                yi_h = yi.rearrange("p (h e) -> p h e", h=H)
                for dup in range(2):
                    nc.scalar.copy(vfr[:, :, dup, :], yr_h)
                    nc.scalar.copy(vfi[:, :, dup, :], yi_h)
                V2 = v2_pool.tile([128, H, 2, PT], BF16, tag="V2")
                nc.sync.dma_start_transpose(
                    V2[:, :, 0, :], vfr[:].rearrange("p a b c -> p (a b c)"))
                nc.sync.dma_start_transpose(
                    V2[:, :, 1, :], vfi[:].rearrange("p a b c -> p (a b c)"))
                v6 = v6_pool.tile([128, H, NJQ, NPJ, 6], BF16, tag="v6")
                v6b = v6_pool.tile([128, H, NJQ, NPJ, 6], BF16, tag="v6b")
                for c in range(2):
                    tv = V2[:, :, c, :].rearrange(
                        "p h (a blk b) -> p h a blk b", a=NJQ, blk=2)
                    nc.gpsimd.tensor_copy(v6[0:64, :, :, :, c],
                                          tv[0:64, :, :, 0, :])
                    nc.gpsimd.tensor_copy(v6[64:128, :, :, :, 3 + c],
                                          tv[64:128, :, :, 1, :])
                    nc.gpsimd.tensor_scalar(v6b[0:64, :, :, :, c],
                                            tv[0:64, :, :, 0, :],
                                            INV_SQRT2, None, op0=ALU.mult)
                    nc.gpsimd.tensor_scalar(v6b[64:128, :, :, :, 3 + c],
                                            tv[64:128, :, :, 1, :],
                                            INV_SQRT2, None, op0=ALU.mult)
                v2_of[tt] = V2
                v6_of[tt] = (v6, v6b)

            # pre-zero the staging HS buffer's zero-slots (A rows carry
            # data in blk 0 slots, B rows in blk 1; the complement stays 0)
            hs0 = stage_pool.tile([4, NPJ, H, 2, E], BF16, name="hs0",
                                  tag="HS")
            nc.vector.memset(hs0[:], 0.0)
            # preset v6 zero and ones slots on both rotating buffers
            for _ in range(2):
                for tg in ("v6", "v6b"):
                    one = 1.0 if tg == "v6" else INV_SQRT2
                    t6 = v6_pool.tile([128, H, NJQ, NPJ, 6], BF16,
                                      name=f"pre_{tg}", tag=tg)
                    nc.vector.memset(t6[0:64, :, :, :, 3:6], 0.0)
                    nc.vector.memset(t6[64:128, :, :, :, 0:3], 0.0)
                    nc.vector.memset(t6[0:64, :, :, :, 2], one)
                    nc.vector.memset(t6[64:128, :, :, :, 5], one)

            # ---- attention, with next-tt projections emitted ahead ----
            emit_qk("q", 0)
            emit_qk("k", 0)
            emit_v(0)
            def emit_staging(stt, sjq):
                arng = slice(sjq * TQ, sjq * TQ + NPJ)
                brng = slice(sjq * TQ + NPJ, sjq * TQ + TQ)
                Gn = gn_of[stt]
                Hn = hn_of[stt]
                GS = gs_pool.tile([4, NPJ, H, E], BF16, name="GS", tag="GS")
                HS = stage_pool.tile([4, NPJ, H, 2, E], BF16, name="HS",
                                     tag="HS")
                for r, (rng, ty) in enumerate(
                        ((arng, 0), (arng, 1), (brng, 0), (brng, 1))):
                    nc.sync.dma_start(GS[r:r + 1, :, :, :],
                                      Gn[rng, ty, :, :])
                    blk = r // 2
                    nc.sync.dma_start(HS[r:r + 1, :, :, blk, :],
                                      Hn[rng, ty, :, :])
                st_of[(stt, sjq)] = (GS, HS)

            st_of = {}
            emit_staging(0, 0)
            for tt in range(NTT):
                V2 = v2_of.pop(tt)
                v6, v6b = v6_of.pop(tt)
                for jq in range(NJQ):
                    # issue next quarter's staging ahead of everything else
                    njq = (jq + 1) % NJQ
                    ntt = tt + (1 if njq == 0 else 0)
                    if ntt < NTT and (tt + 1 < NTT or njq != 0):
                        if (ntt, njq) == (tt + 1, 0):
                            pass  # deferred below until Gn/Hn exist
                        else:
                            emit_staging(ntt, njq)
                    if tt + 1 < NTT:
                        if jq == 1:
                            emit_qk("q", tt + 1)
                        elif jq == 2:
                            emit_qk("k", tt + 1)
                            emit_staging(tt + 1, 0)
                        elif jq == 3:
                            emit_v(tt + 1)
                    GS, HS = st_of.pop((tt, jq))

                    ctr = ctr_pool.tile([128, KT, TQ, 3], F32, tag="ctr")
                    S = None
                    for h in range(H):
                        par = h % 2
                        k = h // 2
                        ab = ps_pool.tile([128, NPJ, E], F32, tag="ps")
                        for j in range(NPJ):
                            nc.tensor.matmul(ab[:, j, :],
                                             HS[0:4, j, h, :, :],
                                             GS[0:4, j, h, :])
                        if par == 0:
                            S = s_pool.tile([128, 2, NPJ, E], F16, tag="S")
                        nc.scalar.activation(
                            S[:, par, :, :].rearrange("p a b -> p (a b)"),
                            ab[:].rearrange("p a b -> p (a b)"),
                            AF.Sqrt, scale=SQ_SCALE)
                        if par == 0:
                            continue
                        Et1 = et_pool.tile([128, 2, NPJ, E], BF16, tag="E1")
                        Et2 = et_pool.tile([128, 2, NPJ, E], BF16, tag="E2")
                        sflat = S[:].rearrange("p a b c -> p (a b c)")
                        nc.vector.tensor_scalar(
                            Et1[:].rearrange("p a b c -> p (a b c)").bitcast(I16),
                            sflat, BPRIME, None, op0=ALU.add)
                        nc.vector.tensor_scalar(
                            Et2[:].rearrange("p a b c -> p (a b c)").bitcast(I16),
                            Et1[:].rearrange("p a b c -> p (a b c)").bitcast(I16),
                            64.0, None, op0=ALU.add)
                        for hh in (h - 1, h):
                            pp = hh % 2
                            base = 64 * pp
                            cv = ctr[base:base + 64, k, :, :].rearrange(
                                "p (blk jl) c -> p jl blk c", blk=2)
                            for j in range(NPJ):
                                dst = cv[:, j, :, :]
                                nc.tensor.matmul(
                                    dst, Et1[:, pp, j, :],
                                    v6[:, hh, jq, j, :],
                                    start=True, stop=False)
                                nc.tensor.matmul(
                                    dst, Et2[:, pp, j, :],
                                    v6b[:, hh, jq, j, :],
                                    start=False, stop=True)

                    # normalize + write o-proj operands
                    rcp = norm_pool.tile([128, KT, TQ], F32, tag="rcp")
                    nc.vector.reciprocal(rcp[:], ctr[:, :, :, 2])
                    tsl = slice(tt * PT + jq * TQ, tt * PT + (jq + 1) * TQ)
                    nc.vector.tensor_mul(VrT[:, :, tsl], ctr[:, :, :, 0],
                                         rcp[:])
                    nc.vector.tensor_mul(ViT[:, :, tsl], ctr[:, :, :, 1],
                                         rcp[:])

            # ---- phase 4: output projection ----
            wo = {c: load_kxn(w_pool, w["o", c], f"w{c}", D, eng=nc.sync)
                  for c in ("r", "i", "in")}
            wo["br"] = bs["o", "r"]
            wo["bi"] = bs["o", "i"]
            for tt in range(NTT):
                ts = bass.ts(tt, PT)
                ps = ps_pool.tile([128, NPJ, E], F32, tag="ps")
                flat = ps[:].rearrange("p a b -> p (a b)")
                our = flat[:, 0:D]
                oui = flat[:, D:2 * D]
                for k in range(KT):
                    nc.tensor.matmul(our, VrT[:, k, ts], wo["r"][:, k, :],
                                     start=(k == 0), stop=False)
                for k in range(KT):
                    nc.tensor.matmul(our, ViT[:, k, ts], wo["in"][:, k, :],
                                     start=False, stop=False)
                nc.tensor.matmul(our, ones_row[:], wo["br"],
                                 start=False, stop=True)
                for k in range(KT):
                    nc.tensor.matmul(oui, ViT[:, k, ts], wo["r"][:, k, :],
                                     start=(k == 0), stop=False)
                for k in range(KT):
                    nc.tensor.matmul(oui, VrT[:, k, ts], wo["i"][:, k, :],
                                     start=False, stop=False)
                nc.tensor.matmul(oui, ones_row[:], wo["bi"],
                                 start=False, stop=True)

                sor = evac_pool.tile([PT, D], F32, tag="sor")
                soi = evac_pool.tile([PT, D], F32, tag="soi")
                nc.scalar.copy(sor[:], our)
                nc.scalar.copy(soi[:], oui)
                nc.sync.dma_start(out_r[ts, :], sor[:])
                nc.sync.dma_start(out_i[ts, :], soi[:])

    nc.compile()
    return nc


_NC_CACHE = None


def _get_module():
    global _NC_CACHE
    if _NC_CACHE is None:
        _NC_CACHE = _build_module()
    return _NC_CACHE


def _prep_inputs(inputs):
    """host-side shard/layout prep -> list of 8 per-core input maps"""
    import ml_dtypes
    bf = ml_dtypes.bfloat16
    TT = B * L
    xs = {nm: np.ascontiguousarray(
        np.asarray(inputs[nm]).reshape(TT, D).T.astype(bf))
        for nm in ("q_r", "q_i", "k_r", "k_i", "v_r", "v_i")}
    # h-major permutation d' = h*64 + l  ->  natural col l*H + h
    perm = np.empty(D, np.int64)
    for h in range(H):
        for l in range(E):
            perm[h * E + l] = l * H + h
    common = {}
    for p in ("q", "k", "v", "o"):
        wr = np.asarray(inputs[f"w{p}_r"]).astype(np.float32)
        wi = np.asarray(inputs[f"w{p}_i"]).astype(np.float32)
        br = np.asarray(inputs[f"b{p}_r"]).astype(np.float32)
        bi = np.asarray(inputs[f"b{p}_i"]).astype(np.float32)
        wrT = wr.T
        wiT = wi.T
        bm = br - bi
        bp = br + bi
        if p in ("q", "k"):
            # projection emits tp = yr+yi and -(yr-yi):
            #   tp = xr@(wr+wi).T + xi@(wr-wi).T + 2 br
            #   tm'= xi@(wr+wi).T - xr@(wr-wi).T + 2 bi   (= -(yr-yi))
            wsum = wrT + wiT
            wdif = wrT - wiT
            common[f"w_{p}_r"] = np.ascontiguousarray(wsum.astype(bf))
            common[f"w_{p}_in"] = np.ascontiguousarray(wdif.astype(bf))
            common[f"w_{p}_i"] = np.ascontiguousarray((-wdif).astype(bf))
            common[f"b_{p}_r"] = (2 * br).reshape(1, D).astype(bf)
            common[f"b_{p}_i"] = (2 * bi).reshape(1, D).astype(bf)
            continue
        if p == "o":
            # o-proj contracts over h-major d': permute weight rows
            wrT = wrT[perm, :]
            wiT = wiT[perm, :]
        if p == "v":
            # v-proj emits h-major cols: permute weight cols + bias
            wrT = wrT[:, perm]
            wiT = wiT[:, perm]
            bm = bm[perm]
            bp = bp[perm]
        common[f"w_{p}_r"] = np.ascontiguousarray(wrT.astype(bf))
        common[f"w_{p}_i"] = np.ascontiguousarray(wiT.astype(bf))
        common[f"w_{p}_in"] = np.ascontiguousarray((-wiT).astype(bf))
        common[f"b_{p}_r"] = bm.reshape(1, D).astype(bf)
        common[f"b_{p}_i"] = bp.reshape(1, D).astype(bf)
    maps = []
    for c in range(NCORES):
        m = dict(common)
        sl = slice(c * T, (c + 1) * T)
        for nm, arr in xs.items():
            m[f"x_{nm}_T"] = np.ascontiguousarray(arr[:, sl])
        maps.append(m)
    return maps


def kernel(**inputs):
    nc = _get_module()
    maps = _prep_inputs(inputs)
    res = run_bass_kernel_spmd(nc, maps, core_ids=list(range(NCORES)))
    out_r = np.concatenate([res.results[c]["out_r"] for c in range(NCORES)],
                           axis=0).reshape(B, L, D)
    out_i = np.concatenate([res.results[c]["out_i"] for c in range(NCORES)],
                           axis=0).reshape(B, L, D)
    return out_r, out_i


# revision 31
# speedup vs baseline: 2.6136x; 1.0389x over previous
"""ComplexAttentionLayer Trainium2 kernel, v3 (8-core data-parallel).

Math (per token t, head h; E=64; per-head feature dim is 1, so scores are
outer products over the E axis):
  w[l,s]   = Gp[l]*Hm[s] + Gm[l]*Hp[s]       (= 2*abs2, PE outer products)
             Gp=(qr+qi)^2, Gm=(qr-qi)^2, Hp=(kr+ki)^2, Hm=(kr-ki)^2
  score    = sqrt(0.5*w)                      (ACT Sqrt table, exact)
  E        = exp(score)   via the Schraudolph bf16 bit trick on the DVE:
             bits16 = round(A2*score + 16256), A2 = 128/ln2; the bf16 with
             those bits is exp(score)*R(phi), R in [1, 1.0613] a mantissa
             sawtooth.  A second sample bits16+64 shifts the sawtooth phase
             by half a period (and multiplies by sqrt2); contracting
             E1 against V and E2 against V/sqrt2 in one accumulating PSUM
             group averages the two phases: residual error ~ +-0.8%.
  out[l]   = sum_s E[l,s] v[s] / sum_s E[l,s]  (PE per-token matmuls with a
             ones column for the denominator; DVE reciprocal+mul normalize)

The ACT engine runs ONLY the sqrt pass (one table set, loaded once); the
exp lives on the DVE at its 4x (2-byte) rate; abs2/contraction/projections
are PE matmuls; staging uses 8 flatten-DMAs per (tt, quarter) and V is
transposed with the XBAR dma_start_transpose.
"""

import math

import numpy as np

import concourse.bass as bass
import concourse.tile as tile
from concourse import bacc, mybir
from concourse.bass_utils import run_bass_kernel_spmd

AF = mybir.ActivationFunctionType
ALU = mybir.AluOpType
F32 = mybir.dt.float32
F16 = mybir.dt.float16
I16 = mybir.dt.int16
BF16 = mybir.dt.bfloat16

B, L, D, H = 4, 1024, 512, 8
E = D // H           # 64
NCORES = 8
T = B * L // NCORES  # 512 tokens per core
PT = 128             # tokens per tile
NTT = T // PT        # 4 token tiles per core
KT = D // 128        # 4 k-tiles per weight
NJQ = 4              # token quarters per tile
TQ = PT // NJQ       # 32 tokens per (tt, jq)
NPJ = TQ // 2        # 16 pairs per (tt, jq)

A2 = 128.0 / math.log(2.0)
SQ_SCALE = 0.5 * A2 * A2   # sqrt(SQ_SCALE*w) = A2*sqrt(0.5*w) = A2*score
BPRIME = 16256.0
INV_SQRT2 = 1.0 / math.sqrt(2.0)


def _build_module():
    nc = bacc.Bacc()

    xT = {}
    for nm in ("q_r", "q_i", "k_r", "k_i", "v_r", "v_i"):
        xT[nm] = nc.declare_dram_parameter(f"x_{nm}_T", [D, T], BF16, isOutput=False)
    w = {}
    for p in ("q", "k", "v", "o"):
        for c in ("r", "i", "in"):  # r = w_r.T, i = w_i.T, in = -w_i.T
            w[p, c] = nc.declare_dram_parameter(f"w_{p}_{c}", [D, D], BF16,
                                                isOutput=False)
    bias = {}
    for p in ("q", "k", "v", "o"):
        for c in ("r", "i"):  # r: br-bi, i: br+bi
            bias[p, c] = nc.declare_dram_parameter(f"b_{p}_{c}", [1, D], BF16,
                                                   isOutput=False)
    out_r = nc.declare_dram_parameter("out_r", [T, D], F32, isOutput=True)
    out_i = nc.declare_dram_parameter("out_i", [T, D], F32, isOutput=True)

    with tile.TileContext(nc) as tc:
        with (
            tc.tile_pool(name="const", bufs=1) as const_pool,
            tc.tile_pool(name="xin", bufs=1) as x_pool,
            tc.tile_pool(name="wgt", bufs=1) as w_pool,
            tc.tile_pool(name="stage", bufs=1) as stage_pool,
            tc.tile_pool(name="gstage", bufs=2) as gs_pool,
            tc.tile_pool(name="v2", bufs=1) as v2_pool,
            tc.tile_pool(name="v6", bufs=2) as v6_pool,
            tc.tile_pool(name="gh", bufs=2) as gh_pool,
            tc.tile_pool(name="evac", bufs=1) as evac_pool,
            tc.tile_pool(name="sco", bufs=2) as s_pool,
            tc.tile_pool(name="et", bufs=1) as et_pool,
            tc.tile_pool(name="nrm", bufs=1) as norm_pool,
            tc.tile_pool(name="ps", bufs=3, space="PSUM") as ps_pool,
            tc.tile_pool(name="psc", bufs=2, space="PSUM") as ctr_pool,
        ):
            ones_row = const_pool.tile([1, 128], BF16, tag="ones")
            nc.gpsimd.memset(ones_row[:], 1.0)

            ball = const_pool.tile([1, 8, D], BF16, tag="ball")
            bs = {}


            # o-projection operands (h-major d' = h*64+l)
            VrT = const_pool.tile([128, KT, T], BF16, tag="VrT")
            ViT = const_pool.tile([128, KT, T], BF16, tag="ViT")

            def load_kxn(pool, dram, tag, n, eng=None):
                t = pool.tile([128, KT, n], BF16, name=tag, tag=tag)
                (eng or nc.gpsimd).dma_start(
                    t[:], dram[:].rearrange("(k p) n -> p k n", p=128))
                return t

            def cproj(wset, xr_t, xi_t, tt):
                """complex linear on token tile tt -> psum [128, 1024]
                (yr cols 0:512, yi cols 512:1024)"""
                ts = slice(0, PT)
                ps = ps_pool.tile([128, NPJ, E], F32, tag="ps")
                flat = ps[:].rearrange("p a b -> p (a b)")
                yr = flat[:, 0:D]
                yi = flat[:, D:2 * D]
                for k in range(KT):
                    nc.tensor.matmul(yr, xr_t[:, k, ts], wset["r"][:, k, :],
                                     start=(k == 0), stop=False)
                for k in range(KT):
                    nc.tensor.matmul(yr, xi_t[:, k, ts], wset["in"][:, k, :],
                                     start=False, stop=False)
                nc.tensor.matmul(yr, ones_row[:], wset["br"],
                                 start=False, stop=True)
                for k in range(KT):
                    nc.tensor.matmul(yi, xi_t[:, k, ts], wset["r"][:, k, :],
                                     start=(k == 0), stop=False)
                for k in range(KT):
                    nc.tensor.matmul(yi, xr_t[:, k, ts], wset["i"][:, k, :],
                                     start=False, stop=False)
                nc.tensor.matmul(yi, ones_row[:], wset["bi"],
                                 start=False, stop=True)
                return ps, yr, yi

            def nat(ap):
                # [p, (l h)] natural projection cols -> [p, l, h]
                return ap.rearrange("p (l h) -> p l h", h=H)

            # ---- projections, software-pipelined per tt ----
            def load_proj(p):
                wt = p if p != "o" else "q"
                ws = {c: load_kxn(w_pool, w[p, c], f"w{wt}{c}", D,
                                  eng=nc.sync)
                      for c in ("r", "i", "in")}
                ws["br"] = bs[p, "r"]
                ws["bi"] = bs[p, "i"]
                return ws

            def load_biases():
                for i, (key, dram) in enumerate(sorted(bias.items())):
                    nc.sync.dma_start(ball[:, i, :], dram[:])
                    bs[key] = ball[:, i, :]

            def load_x(p, tt):
                xr_t = x_pool.tile([128, KT, PT], BF16, name=f"x{p}r",
                                   tag=f"x{p}r")
                xi_t = x_pool.tile([128, KT, PT], BF16, name=f"x{p}i",
                                   tag=f"x{p}i")
                sl = bass.ts(tt, PT)
                nc.gpsimd.dma_start(
                    xr_t[:],
                    xT[f"{p}_r"][:].rearrange("(k p) n -> p k n", p=128)[:, :, sl])
                nc.gpsimd.dma_start(
                    xi_t[:],
                    xT[f"{p}_i"][:].rearrange("(k p) n -> p k n", p=128)[:, :, sl])
                return xr_t, xi_t

            # PE p-state warmup: ~3us of junk matmuls while inputs load
            for _ in range(2):
                wps = ps_pool.tile([128, NPJ, E], F32, tag="ps")
                wf = wps[:].rearrange("p a b -> p (a b)")
                for i in range(8):
                    nc.tensor.matmul(wf[:, bass.ts(i, 128)], ones_row[:],
                                     ones_row[:])

            prj = {}
            for pp_ in ("q", "k", "v"):
                prj[pp_] = None  # placeholder, filled below in load order
            # q weights first (they gate the first projection), then biases,
            # then the rest
            ws_q = {c: load_kxn(w_pool, w["q", c], f"wq{c}", D, eng=nc.sync)
                    for c in ("r", "i", "in")}
            load_biases()
            ws_q["br"] = bs["q", "r"]
            ws_q["bi"] = bs["q", "i"]
            prj["q"] = ws_q
            for pp_ in ("k", "v"):
                prj[pp_] = load_proj(pp_)
            v2_of = {}
            v6_of = {}
            gn_of = {}
            hn_of = {}

            def emit_qk(p, tt):
                # host-prepped weight combos make the projection emit
                # yr+yi (yr slot) and -(yr-yi) (yi slot) directly; one ACT
                # Square per half evacuates PSUM into Gn/Hn (sign drops)
                ws = prj[p]
                xr_t, xi_t = load_x(p, tt)
                ps, yr, yi = cproj(ws, xr_t, xi_t, tt)
                if p == "q":
                    dst = gh_pool.tile([PT, 2, H, E], BF16, name="Gn",
                                       tag="Gn")
                    gn_of[tt] = dst
                else:
                    dst = gh_pool.tile([PT, 2, H, E], BF16, name="Hn",
                                       tag="Hn")
                    hn_of[tt] = dst
                ty_p = 0 if p == "q" else 1
                ty_m = 1 - ty_p
                nc.scalar.activation(
                    dst[:, ty_p, :, :].rearrange("p h l -> p l h"),
                    nat(yr), AF.Square)
                nc.scalar.activation(
                    dst[:, ty_m, :, :].rearrange("p h l -> p l h"),
                    nat(yi), AF.Square)

            def emit_v(tt):
                ws = prj["v"]
                xr_t, xi_t = load_x("v", tt)
                ps, yr, yi = cproj(ws, xr_t, xi_t, tt)
                vfr = evac_pool.tile([128, H, 2, E], BF16, tag="vfr")
                vfi = evac_pool.tile([128, H, 2, E], BF16, tag="vfi")
                yr_h = yr.rearrange("p (h e) -> p h e", h=H)
                yi_h = yi.rearrange("p (h e) -> p h e", h=H)
                for dup in range(2):
                    nc.scalar.copy(vfr[:, :, dup, :], yr_h)
                    nc.scalar.copy(vfi[:, :, dup, :], yi_h)
                V2 = v2_pool.tile([128, H, 2, PT], BF16, tag="V2")
                nc.sync.dma_start_transpose(
                    V2[:, :, 0, :], vfr[:].rearrange("p a b c -> p (a b c)"))
                nc.sync.dma_start_transpose(
                    V2[:, :, 1, :], vfi[:].rearrange("p a b c -> p (a b c)"))
                v6 = v6_pool.tile([128, H, NJQ, NPJ, 6], BF16, tag="v6")
                v6b = v6_pool.tile([128, H, NJQ, NPJ, 6], BF16, tag="v6b")
                for c in range(2):
                    tv = V2[:, :, c, :].rearrange(
                        "p h (a blk b) -> p h a blk b", a=NJQ, blk=2)
                    nc.gpsimd.tensor_copy(v6[0:64, :, :, :, c],
                                          tv[0:64, :, :, 0, :])
                    nc.gpsimd.tensor_copy(v6[64:128, :, :, :, 3 + c],
                                          tv[64:128, :, :, 1, :])
                    nc.gpsimd.tensor_scalar(v6b[0:64, :, :, :, c],
                                            tv[0:64, :, :, 0, :],
                                            INV_SQRT2, None, op0=ALU.mult)
                    nc.gpsimd.tensor_scalar(v6b[64:128, :, :, :, 3 + c],
                                            tv[64:128, :, :, 1, :],
                                            INV_SQRT2, None, op0=ALU.mult)
                v2_of[tt] = V2
                v6_of[tt] = (v6, v6b)

            # pre-zero the staging HS buffer's zero-slots (A rows carry
            # data in blk 0 slots, B rows in blk 1; the complement stays 0)
            hs0 = stage_pool.tile([4, NPJ, H, 2, E], BF16, name="hs0",
                                  tag="HS")
            nc.vector.memset(hs0[:], 0.0)
            # preset v6 zero and ones slots on both rotating buffers
            for _ in range(2):
                for tg in ("v6", "v6b"):
                    one = 1.0 if tg == "v6" else INV_SQRT2
                    t6 = v6_pool.tile([128, H, NJQ, NPJ, 6], BF16,
                                      name=f"pre_{tg}", tag=tg)
                    nc.vector.memset(t6[0:64, :, :, :, 3:6], 0.0)
                    nc.vector.memset(t6[64:128, :, :, :, 0:3], 0.0)
                    nc.vector.memset(t6[0:64, :, :, :, 2], one)
                    nc.vector.memset(t6[64:128, :, :, :, 5], one)

            # ---- attention, with next-tt projections emitted ahead ----
            emit_qk("q", 0)
            emit_qk("k", 0)
            emit_v(0)
            def emit_staging(stt, sjq):
                arng = slice(sjq * TQ, sjq * TQ + NPJ)
                brng = slice(sjq * TQ + NPJ, sjq * TQ + TQ)
                Gn = gn_of[stt]
                Hn = hn_of[stt]
                GS = gs_pool.tile([4, NPJ, H, E], BF16, name="GS", tag="GS")
                HS = stage_pool.tile([4, NPJ, H, 2, E], BF16, name="HS",
                                     tag="HS")
                rows = ((arng, 0), (arng, 1), (brng, 0), (brng, 1))
                for r, (rng, ty) in enumerate(rows):
                    nc.sync.dma_start(GS[r:r + 1, :, :, :],
                                      Gn[rng, ty, :, :])
                for r, (rng, ty) in enumerate(rows):
                    nc.sync.dma_start(HS[r:r + 1, :, :, r // 2, :],
                                      Hn[rng, ty, :, :])
                st_of[(stt, sjq)] = (GS, HS)

            st_of = {}
            emit_staging(0, 0)
            for tt in range(NTT):
                V2 = v2_of.pop(tt)
                v6, v6b = v6_of.pop(tt)
                for jq in range(NJQ):
                    # issue next quarter's staging ahead of everything else
                    njq = (jq + 1) % NJQ
                    ntt = tt + (1 if njq == 0 else 0)
                    if ntt < NTT and (tt + 1 < NTT or njq != 0):
                        if (ntt, njq) == (tt + 1, 0):
                            pass  # deferred below until Gn/Hn exist
                        else:
                            emit_staging(ntt, njq)
                    if tt + 1 < NTT:
                        if jq == 1:
                            emit_qk("q", tt + 1)
                        elif jq == 2:
                            emit_qk("k", tt + 1)
                            emit_staging(tt + 1, 0)
                        elif jq == 3:
                            emit_v(tt + 1)
                    GS, HS = st_of.pop((tt, jq))

                    ctr = ctr_pool.tile([128, KT, TQ, 3], F32, tag="ctr")
                    S = None
                    for h in range(H):
                        par = h % 2
                        k = h // 2
                        ab = ps_pool.tile([128, NPJ, E], F32, tag="ps")
                        for j in range(NPJ):
                            nc.tensor.matmul(ab[:, j, :],
                                             HS[0:4, j, h, :, :],
                                             GS[0:4, j, h, :])
                        if par == 0:
                            S = s_pool.tile([128, 2, NPJ, E], F16, tag="S")
                        nc.scalar.activation(
                            S[:, par, :, :].rearrange("p a b -> p (a b)"),
                            ab[:].rearrange("p a b -> p (a b)"),
                            AF.Sqrt, scale=SQ_SCALE)
                        if par == 0:
                            continue
                        Et1 = et_pool.tile([128, 2, NPJ, E], BF16, tag="E1")
                        Et2 = et_pool.tile([128, 2, NPJ, E], BF16, tag="E2")
                        sflat = S[:].rearrange("p a b c -> p (a b c)")
                        nc.vector.tensor_scalar(
                            Et1[:].rearrange("p a b c -> p (a b c)").bitcast(I16),
                            sflat, BPRIME, None, op0=ALU.add)
                        nc.vector.tensor_scalar(
                            Et2[:].rearrange("p a b c -> p (a b c)").bitcast(I16),
                            Et1[:].rearrange("p a b c -> p (a b c)").bitcast(I16),
                            64.0, None, op0=ALU.add)
                        for hh in (h - 1, h):
                            pp = hh % 2
                            base = 64 * pp
                            cv = ctr[base:base + 64, k, :, :].rearrange(
                                "p (blk jl) c -> p jl blk c", blk=2)
                            for j in range(NPJ):
                                dst = cv[:, j, :, :]
                                nc.tensor.matmul(
                                    dst, Et1[:, pp, j, :],
                                    v6[:, hh, jq, j, :],
                                    start=True, stop=False)
                                nc.tensor.matmul(
                                    dst, Et2[:, pp, j, :],
                                    v6b[:, hh, jq, j, :],
                                    start=False, stop=True)

                    # normalize + write o-proj operands
                    rcp = norm_pool.tile([128, KT, TQ], F32, tag="rcp")
                    nc.vector.reciprocal(rcp[:], ctr[:, :, :, 2])
                    tsl = slice(tt * PT + jq * TQ, tt * PT + (jq + 1) * TQ)
                    nc.vector.tensor_mul(VrT[:, :, tsl], ctr[:, :, :, 0],
                                         rcp[:])
                    nc.vector.tensor_mul(ViT[:, :, tsl], ctr[:, :, :, 1],
                                         rcp[:])

            # ---- phase 4: output projection ----
            wo = {c: load_kxn(w_pool, w["o", c], f"w{c}", D, eng=nc.sync)
                  for c in ("r", "i", "in")}
            wo["br"] = bs["o", "r"]
            wo["bi"] = bs["o", "i"]
            for tt in range(NTT):
                ts = bass.ts(tt, PT)
                ps = ps_pool.tile([128, NPJ, E], F32, tag="ps")
                flat = ps[:].rearrange("p a b -> p (a b)")
                our = flat[:, 0:D]
                oui = flat[:, D:2 * D]
                for k in range(KT):
                    nc.tensor.matmul(our, VrT[:, k, ts], wo["r"][:, k, :],
                                     start=(k == 0), stop=False)
                for k in range(KT):
                    nc.tensor.matmul(our, ViT[:, k, ts], wo["in"][:, k, :],
                                     start=False, stop=False)
                nc.tensor.matmul(our, ones_row[:], wo["br"],
                                 start=False, stop=True)
                for k in range(KT):
                    nc.tensor.matmul(oui, ViT[:, k, ts], wo["r"][:, k, :],
                                     start=(k == 0), stop=False)
                for k in range(KT):
                    nc.tensor.matmul(oui, VrT[:, k, ts], wo["i"][:, k, :],
                                     start=False, stop=False)
                nc.tensor.matmul(oui, ones_row[:], wo["bi"],
                                 start=False, stop=True)

                sor = evac_pool.tile([PT, D], F32, tag="sor")
                soi = evac_pool.tile([PT, D], F32, tag="soi")
                nc.scalar.copy(sor[:], our)
                nc.scalar.copy(soi[:], oui)
                nc.sync.dma_start(out_r[ts, :], sor[:])
                nc.sync.dma_start(out_i[ts, :], soi[:])

    nc.compile()
    return nc


_NC_CACHE = None


def _get_module():
    global _NC_CACHE
    if _NC_CACHE is None:
        _NC_CACHE = _build_module()
    return _NC_CACHE


def _prep_inputs(inputs):
    """host-side shard/layout prep -> list of 8 per-core input maps"""
    import ml_dtypes
    bf = ml_dtypes.bfloat16
    TT = B * L
    xs = {nm: np.ascontiguousarray(
        np.asarray(inputs[nm]).reshape(TT, D).T.astype(bf))
        for nm in ("q_r", "q_i", "k_r", "k_i", "v_r", "v_i")}
    # h-major permutation d' = h*64 + l  ->  natural col l*H + h
    perm = np.empty(D, np.int64)
    for h in range(H):
        for l in range(E):
            perm[h * E + l] = l * H + h
    common = {}
    for p in ("q", "k", "v", "o"):
        wr = np.asarray(inputs[f"w{p}_r"]).astype(np.float32)
        wi = np.asarray(inputs[f"w{p}_i"]).astype(np.float32)
        br = np.asarray(inputs[f"b{p}_r"]).astype(np.float32)
        bi = np.asarray(inputs[f"b{p}_i"]).astype(np.float32)
        wrT = wr.T
        wiT = wi.T
        bm = br - bi
        bp = br + bi
        if p in ("q", "k"):
            # projection emits tp = yr+yi and -(yr-yi):
            #   tp = xr@(wr+wi).T + xi@(wr-wi).T + 2 br
            #   tm'= xi@(wr+wi).T - xr@(wr-wi).T + 2 bi   (= -(yr-yi))
            wsum = wrT + wiT
            wdif = wrT - wiT
            common[f"w_{p}_r"] = np.ascontiguousarray(wsum.astype(bf))
            common[f"w_{p}_in"] = np.ascontiguousarray(wdif.astype(bf))
            common[f"w_{p}_i"] = np.ascontiguousarray((-wdif).astype(bf))
            common[f"b_{p}_r"] = (2 * br).reshape(1, D).astype(bf)
            common[f"b_{p}_i"] = (2 * bi).reshape(1, D).astype(bf)
            continue
        if p == "o":
            # o-proj contracts over h-major d': permute weight rows
            wrT = wrT[perm, :]
            wiT = wiT[perm, :]
        if p == "v":
            # v-proj emits h-major cols: permute weight cols + bias
            wrT = wrT[:, perm]
            wiT = wiT[:, perm]
            bm = bm[perm]
            bp = bp[perm]
        common[f"w_{p}_r"] = np.ascontiguousarray(wrT.astype(bf))
        common[f"w_{p}_i"] = np.ascontiguousarray(wiT.astype(bf))
        common[f"w_{p}_in"] = np.ascontiguousarray((-wiT).astype(bf))
        common[f"b_{p}_r"] = bm.reshape(1, D).astype(bf)
        common[f"b_{p}_i"] = bp.reshape(1, D).astype(bf)
    maps = []
    for c in range(NCORES):
        m = dict(common)
        sl = slice(c * T, (c + 1) * T)
        for nm, arr in xs.items():
            m[f"x_{nm}_T"] = np.ascontiguousarray(arr[:, sl])
        maps.append(m)
    return maps


def kernel(**inputs):
    nc = _get_module()
    maps = _prep_inputs(inputs)
    res = run_bass_kernel_spmd(nc, maps, core_ids=list(range(NCORES)))
    out_r = np.concatenate([res.results[c]["out_r"] for c in range(NCORES)],
                           axis=0).reshape(B, L, D)
    out_i = np.concatenate([res.results[c]["out_i"] for c in range(NCORES)],
                           axis=0).reshape(B, L, D)
    return out_r, out_i
